# revision 42
# baseline (speedup 1.0000x reference)
"""SGC (2-hop simple graph convolution) Trainium2 kernel, 8-core SPMD.

out = S S x W^T + b,  S = D^{-1/2} (A + I) D^{-1/2}   (D = in-degree + 1)

Strategy:
  * project first: y = x @ W^T (64 ch), exact by associativity
  * factor norms:  S z = dinv * (A+I) (dinv * z)  -> per-node scalings only,
    messages are unweighted; self loop handled as a local add
  * per core: own 1/8 of destination nodes; edges partitioned by dst
  * hop-1 table: every core projects ALL nodes (replicated, ~100us of PE) and
    writes the z0 gather table to local DRAM -- no AllGather, no collective
    latency before the first gathers. Each core's own nodes are placed first
    (block rotation) so the z0 extraction for the self-loop add is
    core-independent in the instruction stream.
  * hop-2 table: z1 is AllGather'ed in AGC chunks (chunk-major permutation,
    one Shared tensor per chunk) pipelined behind hop-1 windows
  * each chunk is its own gather "stream" pinned to a SWDGE queue: 4-wide
    descriptor generation on gpsimd, and chunk-s gathers depend only on
    chunk-s data
  * gather via gpsimd dma_gather from bf16 tables (rows padded to 256 B);
    ni<=1024 per call enables single_packet
  * scatter-adds via PE matmul: 128-message tiles x host-built 0/1 one-hot
    stationary tiles (fp8, streamed on the sync HWDGE queue; PE takes fp8
    stationary against bf16 moving), accumulated in PSUM per 128-dst window
  * output staged [p, w, ch]; host un-permutes at the end
"""

import sys

sys.path.insert(0, "/opt/trn_rl_repo")

import numpy as np
import ml_dtypes

# ---------------- problem constants (overridden by tests for small runs) ----
CFG = dict(
    N_NODES=65536,
    N_EDGES=655360,
    IN_C=128,
    OUT_C=64,
    CORES=8,
    CH=8,  # gather tiles (128 msgs each) per dma_gather call; ni<=1024 -> single_packet
    CHP=32,  # one-hot pairs per DMA chunk
    OC_PAD=128,  # bf16 channels per gather-table row (256 B)
    MSG_BUFS=8,  # per-stream message buffers
    OH_BUFS=3,
    PF=4,  # gather chunks prefetched ahead of demand
    PSUM_BUFS=6,
    RESYNC_G=64,  # no intra-stream alignment: fewest gather tiles wins
    ACT_EVAC=1,
    STAGE=6,  # debug: 1 proj, 3 +gather/oh, 4 +hop1 mm, 5 +ag2, 6 full
    AGC=4,  # table chunks == gather streams == SWDGE queues
    QRR=0,  # 1: round-robin gather queue assignment; 0: queue = stream id
)

SENT = 1 << 20  # sentinel "dst" for pad rows -> all-zero one-hot everywhere


class Prep:
    pass


def _row1_of_node(n, me, NP, C, CW):
    # hop-1 table row on core `me` (locally projected, per-core block rotation
    # so own nodes land at iblk=0): row = c*(C*128*CW) + iblk*(128*CW) + p*CW16..
    # layout within a 2048-row block is p-major (p*CW + wr) to match the flat
    # staging DMA order [128p, CW, 256B] -> rows p*CW+wr.
    r = n % NP
    p = r % 128
    w = r // 128
    c = w // CW
    wr = w % CW
    iblk = (n // NP - me) % C
    return c * (C * 128 * CW) + iblk * (128 * CW) + p * CW + wr


def _row2_of_node(n, NP, C, CW):
    # hop-2 table row (AllGather'ed): chunk-major so each AG chunk is one
    # contiguous Shared tensor; within a core block p-major like the staging.
    r = n % NP
    p = r % 128
    w = r // 128
    c = w // CW
    wr = w % CW
    return c * (C * 128 * CW) + (n // NP) * (128 * CW) + p * CW + wr


def _preprocess(edge_index):
    N = CFG["N_NODES"]
    C = CFG["CORES"]
    NP = N // C
    WIN = NP // 128
    AGC = CFG["AGC"]
    CW = WIN // AGC
    CBLK = C * 128 * CW  # rows per chunk
    S = AGC
    assert WIN % AGC == 0
    assert CBLK <= 32768  # int16 gather indices

    src = np.asarray(edge_index[0], dtype=np.int64)
    dst = np.asarray(edge_index[1], dtype=np.int64)
    deg = np.bincount(dst, minlength=N).astype(np.float32) + 1.0

    all_nodes = np.arange(N, dtype=np.int64)
    row_of2 = _row2_of_node(all_nodes, NP, C, CW)

    pr = Prep()
    pr.N, pr.C, pr.NP, pr.WIN = N, C, NP, WIN
    pr.AGC, pr.CW, pr.CBLK, pr.S = AGC, CW, CBLK, S
    pr.deg = deg

    # per-core, per-stream (== per-chunk) sorted message lists
    core_ld = [[None] * S for _ in range(C)]  # local dst per stream
    core_idx1 = [[None] * S for _ in range(C)]  # within-chunk hop-1 table idx
    core_idx2 = [[None] * S for _ in range(C)]  # within-chunk hop-2 table idx
    for i in range(C):
        m = (dst >= i * NP) & (dst < (i + 1) * NP)
        s_i = src[m]
        ld_i = dst[m] - i * NP
        order = np.argsort(ld_i, kind="stable")
        s_i, ld_i = s_i[order], ld_i[order]
        rows1 = _row1_of_node(s_i, i, NP, C, CW)
        rows2 = row_of2[s_i]
        cs = rows2 // CBLK  # chunk id (same for both formulas)
        for s in range(S):
            a = cs == s
            core_ld[i][s] = ld_i[a]
            core_idx1[i][s] = rows1[a] % CBLK
            core_idx2[i][s] = rows2[a] % CBLK

    # re-align all cores' streams at every RESYNC_G windows: within a group,
    # pad each core's segment to the max core's tile count. (G=WIN: only the
    # global per-stream tile count is equalized across cores.)
    G = CFG.get("RESYNC_G", 64)
    n_groups = (WIN + G - 1) // G
    for s in range(S):
        seg_tiles = np.zeros(n_groups, dtype=np.int64)
        for g in range(n_groups):
            lo, hi = g * G * 128, min((g + 1) * G, WIN) * 128
            for i in range(C):
                cnt = int(((core_ld[i][s] >= lo) & (core_ld[i][s] < hi)).sum())
                seg_tiles[g] = max(seg_tiles[g], (cnt + 127) // 128)
        for i in range(C):
            lds, ixs1, ixs2 = [], [], []
            for g in range(n_groups):
                lo, hi = g * G * 128, min((g + 1) * G, WIN) * 128
                m = (core_ld[i][s] >= lo) & (core_ld[i][s] < hi)
                ld_g = core_ld[i][s][m]
                ix1_g, ix2_g = core_idx1[i][s][m], core_idx2[i][s][m]
                pad = int(seg_tiles[g]) * 128 - len(ld_g)
                lds.append(np.concatenate([ld_g, np.full(pad, SENT, np.int64)]))
                ixs1.append(np.concatenate([ix1_g, np.zeros(pad, np.int64)]))
                ixs2.append(np.concatenate([ix2_g, np.zeros(pad, np.int64)]))
            core_ld[i][s] = np.concatenate(lds)
            core_idx1[i][s] = np.concatenate(ixs1)
            core_idx2[i][s] = np.concatenate(ixs2)
    T = [len(core_ld[0][s]) // 128 for s in range(S)]
    pr.T = T

    for i in range(C):
        for s in range(S):
            assert len(core_ld[i][s]) == T[s] * 128

    # union pair structure (w, stream, tile) across cores
    pair_set = set()
    for i in range(C):
        for s in range(S):
            L = core_ld[i][s].reshape(T[s], 128)
            for t in range(T[s]):
                real = L[t][L[t] != SENT]
                if len(real) == 0:
                    continue
                for w in range(int(real.min()) // 128, int(real.max()) // 128 + 1):
                    pair_set.add((w, s, t))
    for w in range(WIN):  # every window needs >=1 pair so psum gets reset
        if not any(p[0] == w for p in pair_set):
            pair_set.add((w, 0, 0))
    pairs = sorted(pair_set)
    pr.pairs = pairs
    pr.n_pairs = len(pairs)
    segs = [[] for _ in range(WIN)]
    for k, (w, s, t) in enumerate(pairs):
        segs[w].append(k)
    pr.segs = segs

    # per-core one-hot tiles [128, n_pairs, 128] fp8e4m3(0/1); the PE takes
    # fp8 stationary against bf16 moving directly, so no cast is needed
    pr.onehot = []
    pr.idx_wrapped = []
    pr.deg_staged = []
    FP8_ONE = 0x38
    for i in range(C):
        oh = np.zeros((128, pr.n_pairs, 128), dtype=np.uint8)
        for k, (w, s, t) in enumerate(pairs):
            ld_t = core_ld[i][s][t * 128 : (t + 1) * 128]
            slot = ld_t - 128 * w
            valid = (slot >= 0) & (slot < 128)
            rr = np.nonzero(valid)[0]
            oh[rr, k, slot[rr]] = FP8_ONE
        pr.onehot.append(oh.view(ml_dtypes.float8_e4m3fn))

        blocks = []
        for core_idx in (core_idx1, core_idx2):
            for s in range(S):
                ix = core_idx[i][s].astype(np.int16)
                assert (core_idx[i][s] < CBLK).all() and (core_idx[i][s] >= 0).all()
                w16 = ix.reshape(-1, 16).T  # [16, T*8]
                blocks.append(np.tile(w16, (8, 1)))  # replicate to 128 partitions
        pr.idx_wrapped.append(
            np.ascontiguousarray(np.concatenate(blocks, axis=1))
        )

        dshard = deg[i * NP : (i + 1) * NP]
        pr.deg_staged.append(
            np.ascontiguousarray(dshard.reshape(WIN, 128).T.astype(np.float32))
        )

    return pr


# ------------------------------------------------------------------ bass ----


def _build(pr):
    import concourse.bass as bass
    import concourse.bacc as bacc
    import concourse.mybir as mybir
    import concourse.tile as tile
    from concourse._compat import get_trn_type

    dt = mybir.dt
    Alu = mybir.AluOpType
    F32, BF16, FP8, I16 = dt.float32, dt.bfloat16, dt.float8e4, dt.int16

    IN_C, OUT_C = CFG["IN_C"], CFG["OUT_C"]
    OC_PAD, CH, CHP = CFG["OC_PAD"], CFG["CH"], CFG["CHP"]
    N, C, NP, WIN = pr.N, pr.C, pr.NP, pr.WIN
    AGC, CW, CBLK, S = pr.AGC, pr.CW, pr.CBLK, pr.S
    CROWS = 128 * CW  # local rows per chunk
    UT = C * WIN  # replicated-projection table windows
    PIECE = 32  # proj windows per streamed xt piece (4096 columns)
    T = pr.T
    NQ = min(S, 4)

    nc = bacc.Bacc(
        get_trn_type() or "TRN2",
        target_bir_lowering=False,
        debug=False,
        num_devices=C,
        num_swdge_queues=NQ,
    )

    xtp_d = nc.dram_tensor("xtp", [IN_C, UT * 128], BF16, kind="ExternalInput")
    wt_d = nc.dram_tensor("wt", [IN_C, OUT_C], BF16, kind="ExternalInput")
    b_d = nc.dram_tensor("bias", [128, OUT_C], F32, kind="ExternalInput")
    deg_d = nc.dram_tensor("deg", [128, WIN], F32, kind="ExternalInput")
    degp_d = nc.dram_tensor("degp", [128, UT], F32, kind="ExternalInput")
    idx_d = nc.dram_tensor(
        "idx", [128, sum(T) * 8 * 2], I16, kind="ExternalInput"
    )
    oh_d = nc.dram_tensor("oh", [128, pr.n_pairs, 128], FP8, kind="ExternalInput")
    out_d = nc.dram_tensor("out", [128, WIN, OUT_C], F32, kind="ExternalOutput")

    rg = [list(range(C))]

    with tile.TileContext(nc) as tc:
        with (
            tc.tile_pool(name="const", bufs=1) as const,
            tc.tile_pool(name="dram", bufs=1, space="DRAM") as dram,
            tc.tile_pool(name="psum_y", bufs=2, space="PSUM") as psum_y,
            tc.tile_pool(name="psum_w", bufs=CFG["PSUM_BUFS"], space="PSUM") as psum_w,
            tc.tile_pool(name="xtb", bufs=2) as xtb_pool,
            tc.tile_pool(name="zsp", bufs=3) as zs_pool,
            tc.tile_pool(name="msg0", bufs=CFG["MSG_BUFS"]) as msg0_pool,
            tc.tile_pool(name="msg1", bufs=CFG["MSG_BUFS"]) as msg1_pool,
            tc.tile_pool(name="msg2", bufs=CFG["MSG_BUFS"]) as msg2_pool,
            tc.tile_pool(name="msg3", bufs=CFG["MSG_BUFS"]) as msg3_pool,
            tc.tile_pool(name="ohp", bufs=CFG["OH_BUFS"]) as oh_pool,
            tc.tile_pool(name="tmp", bufs=4) as tmp_pool,
        ):
            cc2_in = dram.tile([NP, OC_PAD], BF16)
            t1_tabs = [
                dram.tile([CBLK, OC_PAD], BF16, name=f"t1c{c}")
                for c in range(AGC)
            ]
            cc2_outs = [
                dram.tile(
                    [CBLK, OC_PAD], BF16, addr_space="Shared", name=f"cc2o{c}"
                )
                for c in range(AGC)
            ]

            idx_sb = const.tile([128, sum(T) * 8 * 2], I16)
            nc.sync.dma_start(idx_sb[:], idx_d[:])
            wt_sb = const.tile([IN_C, OUT_C], BF16)
            nc.sync.dma_start(wt_sb[:], wt_d[:])
            b_sb = const.tile([128, OUT_C], F32)
            nc.sync.dma_start(b_sb[:], b_d[:])
            deg_sb = const.tile([128, WIN], F32)
            nc.sync.dma_start(deg_sb[:], deg_d[:])
            degp_sb = const.tile([128, UT], F32)
            nc.sync.dma_start(degp_sb[:], degp_d[:])

            deginv = const.tile([128, WIN], F32)
            nc.vector.reciprocal(deginv[:], deg_sb[:])
            dinv = const.tile([128, WIN], F32)
            nc.scalar.activation(
                dinv[:], deginv[:], mybir.ActivationFunctionType.Sqrt
            )
            deginvp = const.tile([128, UT], F32)
            nc.vector.reciprocal(deginvp[:], degp_sb[:])
            dinvp = const.tile([128, UT], F32)
            nc.scalar.activation(
                dinvp[:], deginvp[:], mybir.ActivationFunctionType.Sqrt
            )

            z0f = const.tile([128, WIN, OUT_C], F32)
            z1f = const.tile([128, WIN, OUT_C], F32)
            outst = const.tile([128, WIN, OUT_C], F32)
            zpad2 = const.tile([128, WIN, OC_PAD], BF16)
            nc.vector.memset(zpad2[:], 0.0)

            STAGE = CFG["STAGE"]

            def emit_ag(cc_in, cc_outs, c, zpad):
                nc.sync.dma_start(
                    cc_in[c * CROWS : (c + 1) * CROWS, :],
                    zpad[:, c * CW : (c + 1) * CW, :],
                )
                nc.gpsimd.collective_compute(
                    "AllGather",
                    Alu.bypass,
                    replica_groups=rg,
                    ins=[cc_in[c * CROWS : (c + 1) * CROWS, :].opt()],
                    outs=[cc_outs[c][:].opt()],
                )

            # ---- replicated projection of ALL nodes: z0 table to local DRAM
            # table window u = c*128 + iblk*16.. holds (chunk c, core block
            # iblk, wr = u%CW) across partitions p; iblk==0 is this core.
            xtiles = {}
            zs = None
            zs_count = 0
            for u in range(UT):
                k = u // PIECE
                if u % PIECE == 0:
                    xtiles[k] = xtb_pool.tile(
                        [IN_C, PIECE * 128], BF16, tag="xt", name=f"xt{k % 2}"
                    )
                    nc.scalar.dma_start(
                        xtiles[k][:],
                        xtp_d[:, k * PIECE * 128 : (k + 1) * PIECE * 128],
                    )
                if u % CW == 0:
                    zs = zs_pool.tile([128, CW, OC_PAD], BF16, tag="zs")
                    if zs_count < 3:  # pool bufs cycle; zero the pad lanes once
                        nc.vector.memset(zs[:], 0.0)
                    zs_count += 1
                py = psum_y.tile([128, OUT_C], F32)
                off = (u % PIECE) * 128
                nc.tensor.matmul(
                    py[:],
                    xtiles[k][:, off : off + 128],
                    wt_sb[:],
                    start=True,
                    stop=True,
                )
                if u % 2:
                    nc.scalar.mul(zs[:, u % CW, 0:OUT_C], py[:], dinvp[:, u : u + 1])
                else:
                    nc.vector.tensor_scalar(
                        zs[:, u % CW, 0:OUT_C], py[:], dinvp[:, u : u + 1], None, Alu.mult
                    )
                if (u % 128) < CW:  # iblk == 0: this core's own nodes
                    w = (u // 128) * CW + (u % 128)
                    if u % 2:
                        nc.vector.tensor_scalar(
                            z0f[:, w, :], py[:], dinvp[:, u : u + 1], None, Alu.mult
                        )
                    else:
                        nc.scalar.mul(z0f[:, w, :], py[:], dinvp[:, u : u + 1])
                if u % CW == CW - 1:
                    c = u // 128
                    iblk = (u % 128) // CW
                    nc.scalar.dma_start(
                        t1_tabs[c][iblk * CROWS : (iblk + 1) * CROWS, :], zs[:]
                    )

            calls = [(T[s] + CH - 1) // CH for s in range(S)]
            n_oh_chunks = (pr.n_pairs + CHP - 1) // CHP
            colbase = np.concatenate([[0], np.cumsum(np.array(T) * 8)])
            msg_pools = [msg0_pool, msg1_pool, msg2_pool, msg3_pool][:S]
            gq_counter = [0]

            def run_hop(tabs_list, idx_base, evac, do_mm=True, after_window=None):
                tabs = [t[:] for t in tabs_list]
                msg_tiles = [{} for _ in range(S)]
                oh_tiles = {}
                next_call = [0] * S
                next_oh = [0]

                def emit_gather(s):
                    c = next_call[s]
                    ntiles = min(CH, T[s] - c * CH)
                    ni = ntiles * 128
                    t = msg_pools[s].tile([128, CH, OC_PAD], BF16, tag=f"msg{s}")
                    sl = slice(
                        idx_base + int(colbase[s]) + c * CH * 8,
                        idx_base + int(colbase[s]) + c * CH * 8 + ntiles * 8,
                    )
                    nc.gpsimd.dma_gather(
                        t[:, 0:ntiles, :],
                        tabs[s],
                        idx_sb[:, sl],
                        ni,
                        ni,
                        OC_PAD,
                        single_packet=(ni <= 1024),
                        queue_num=(gq_counter[0] if CFG.get("QRR") else s) % NQ,
                    )
                    gq_counter[0] += 1
                    msg_tiles[s][c] = t
                    next_call[s] = c + 1

                def emit_oh():
                    k = next_oh[0]
                    npair = min(CHP, pr.n_pairs - k * CHP)
                    t = oh_pool.tile([128, CHP, 128], FP8, tag="oh")
                    nc.sync.dma_start(
                        out=t[:, 0:npair, :],
                        in_=oh_d[:, k * CHP : k * CHP + npair, :],
                    )
                    oh_tiles[k] = t
                    next_oh[0] = k + 1

                for w in range(WIN):
                    seg = pr.segs[w]
                    # make sure resources (plus prefetch) exist
                    for pk in seg:
                        _, s, t = pr.pairs[pk]
                        while next_call[s] <= min(
                            t // CH + CFG.get("PF", 1), calls[s] - 1
                        ):
                            emit_gather(s)
                        while next_oh[0] <= min(pk // CHP + 1, n_oh_chunks - 1):
                            emit_oh()
                    if not do_mm:
                        continue
                    pw = psum_w.tile([128, OUT_C], F32)
                    for j, pk in enumerate(seg):
                        _, s, t = pr.pairs[pk]
                        oh_ap = oh_tiles[pk // CHP][:, pk % CHP, :]
                        msg_ap = msg_tiles[s][t // CH][:, t % CH, 0:OUT_C]
                        nc.tensor.matmul(
                            pw[:],
                            oh_ap,
                            msg_ap,
                            start=(j == 0),
                            stop=(j == len(seg) - 1),
                        )
                    evac(w, pw)
                    if after_window is not None:
                        after_window(w)

            # ---- hop 1:  z1 = (psum + z0) / deg ----
            def evac1(w, pw):
                tmp = tmp_pool.tile([128, OUT_C], F32, tag="tmp")
                nc.vector.tensor_add(tmp[:], pw[:], z0f[:, w, :])
                if CFG["ACT_EVAC"]:
                    nc.scalar.mul(z1f[:, w, :], tmp[:], deginv[:, w : w + 1])
                    nc.scalar.mul(
                        zpad2[:, w, 0:OUT_C], tmp[:], deginv[:, w : w + 1]
                    )
                else:
                    nc.vector.tensor_scalar(
                        z1f[:, w, :], tmp[:], deginv[:, w : w + 1], None, Alu.mult
                    )
                    nc.vector.tensor_copy(zpad2[:, w, 0:OUT_C], z1f[:, w, :])

            def after1(w):
                if STAGE >= 5 and (w + 1) % CW == 0:
                    emit_ag(cc2_in, cc2_outs, w // CW, zpad2)

            if STAGE >= 3:
                run_hop(t1_tabs, 0, evac1, do_mm=STAGE >= 4, after_window=after1)

            # ---- hop 2:  out = dinv * (psum + z1) + b ----
            def evac2(w, pw):
                tmp = tmp_pool.tile([128, OUT_C], F32, tag="tmp")
                tmp2 = tmp_pool.tile([128, OUT_C], F32, tag="tmp2")
                nc.vector.tensor_add(tmp[:], pw[:], z1f[:, w, :])
                if CFG["ACT_EVAC"]:
                    nc.scalar.mul(tmp2[:], tmp[:], dinv[:, w : w + 1])
                else:
                    nc.vector.tensor_scalar(
                        tmp2[:], tmp[:], dinv[:, w : w + 1], None, Alu.mult
                    )
                nc.vector.tensor_add(outst[:, w, :], tmp2[:], b_sb[:])

            def after2(w):
                if (w + 1) % CW == 0:
                    c = w // CW
                    nc.sync.dma_start(
                        out_d[:, c * CW : (c + 1) * CW, :],
                        outst[:, c * CW : (c + 1) * CW, :],
                    )

            if STAGE >= 6:
                run_hop(cc2_outs, sum(T) * 8, evac2, after_window=after2)
            else:
                src_final = {1: z0f, 3: z0f, 4: z1f, 5: z1f}[STAGE]
                nc.sync.dma_start(out_d[:], src_final[:])

    nc.compile()
    return nc


def _make_in_maps(pr, x, W, b):
    C, NP, WIN, CW = pr.C, pr.NP, pr.WIN, pr.CW
    UT = C * WIN
    x = np.asarray(x, dtype=np.float32)
    W = np.asarray(W, dtype=np.float32)
    b = np.asarray(b, dtype=np.float32)
    wt = np.ascontiguousarray(W.T.astype(ml_dtypes.bfloat16))
    b_rep = np.ascontiguousarray(np.broadcast_to(b, (128, len(b))))

    # table-window column order for the replicated projection, per core:
    # u = c*128 + iblk*CW + wr, node = ((me+iblk)%C)*NP + (c*CW+wr)*128 + p
    u = np.arange(UT, dtype=np.int64)
    cc = u // 128
    iblk = (u % 128) // CW
    wr = u % CW
    in_maps = []
    for i in range(C):
        owner = (i + iblk) % C
        base = owner * NP + (cc * CW + wr) * 128  # [UT]
        node_of_col = (base[:, None] + np.arange(128)[None, :]).reshape(-1)
        xtp = np.ascontiguousarray(
            x[node_of_col].T.astype(ml_dtypes.bfloat16)
        )
        degp = np.ascontiguousarray(
            pr.deg[node_of_col].reshape(UT, 128).T.astype(np.float32)
        )
        in_maps.append(
            dict(
                xtp=xtp,
                wt=wt,
                bias=b_rep,
                deg=pr.deg_staged[i],
                degp=degp,
                idx=pr.idx_wrapped[i],
                oh=pr.onehot[i],
            )
        )
    return in_maps


def _unpermute(o, pr):
    # device layout is [p, w, ch]; node order is w*128+p
    return (
        o.reshape(128, pr.WIN, o.shape[-1])
        .transpose(1, 0, 2)
        .reshape(pr.NP, o.shape[-1])
    )


def kernel(x, edge_index, W, b):
    pr = _preprocess(edge_index)
    nc = _build(pr)
    in_maps = _make_in_maps(pr, x, W, b)

    from concourse import bass_utils

    res = bass_utils.run_bass_kernel_spmd(
        nc, in_maps, core_ids=list(range(pr.C))
    )
    shards = [_unpermute(res.results[i]["out"], pr) for i in range(pr.C)]
    return np.ascontiguousarray(np.concatenate(shards, axis=0))


# revision 43
# speedup vs baseline: 1.1979x; 1.1979x over previous
"""SGC (2-hop simple graph convolution) Trainium2 kernel, 8-core SPMD.

out = S S x W^T + b,  S = D^{-1/2} (A + I) D^{-1/2}   (D = in-degree + 1)

Strategy:
  * project first: y = x @ W^T (64 ch), exact by associativity
  * factor norms:  S z = dinv * (A+I) (dinv * z)  -> per-node scalings only,
    messages are unweighted; self loop handled as a local add
  * per core: own 1/8 of destination nodes; edges partitioned by dst
  * z tables are AllGather'ed in AGC chunks (chunk-major node permutation so
    every chunk is a contiguous Shared tensor); each chunk is its own gather
    "stream" pinned to its own SWDGE queue, so chunk-s gathers start as soon
    as chunk-s AG lands and descriptor generation runs 4-wide on gpsimd
  * gather sources with gpsimd dma_gather from the bf16 chunk tables
    (rows padded to 128 ch = 256 B to satisfy the elem%256 constraint);
    ni<=1024 per call enables single_packet
  * scatter-adds via PE matmul: 128-message tiles x host-built 0/1 one-hot
    stationary tiles (fp8, streamed on the sync HWDGE queue; the PE takes fp8
    stationary against bf16 moving directly), accumulated in PSUM per
    128-destination window; out-of-window slots give all-zero rows so stream
    tiles may straddle windows with no padding
  * node numbering inside tables is chunk-major permuted; the output staging
    keeps the [p, w] permutation and the host un-permutes at the end
"""

import sys

sys.path.insert(0, "/opt/trn_rl_repo")

import numpy as np
import ml_dtypes

# ---------------- problem constants (overridden by tests for small runs) ----
CFG = dict(
    N_NODES=65536,
    N_EDGES=655360,
    IN_C=128,
    OUT_C=64,
    CORES=8,
    CH=8,  # gather tiles (128 msgs each) per dma_gather call; ni<=1024 -> single_packet
    CHP=32,  # one-hot pairs per DMA chunk
    OC_PAD=128,  # bf16 channels per gather-table row (256 B)
    MSG_BUFS=8,  # per-stream message buffers
    OH_BUFS=3,
    PF=4,  # gather chunks prefetched ahead of demand
    PSUM_BUFS=6,
    RESYNC_G=64,  # no intra-stream alignment: fewest gather tiles wins
    ACT_EVAC=1,
    STAGE=6,  # debug: 1 proj, 2 +ag1, 3 +gather/oh, 4 +hop1 mm, 5 +ag2, 6 full
    AGC=4,  # AllGather chunks == gather streams == SWDGE queues
    QRR=0,  # 1: round-robin gather queue assignment; 0: queue = stream id
)

SENT = 1 << 20  # sentinel "dst" for pad rows -> all-zero one-hot everywhere


class Prep:
    pass


def _row_of_node(n, NP, WIN, C, CW):
    # node n -> chunk-major gather-table row so each AG chunk is contiguous:
    # row = c*(C*128*CW) + core*(128*CW) + p*CW + w%CW   (p = r%128, w = r//128)
    r = n % NP
    p = r % 128
    w = r // 128
    c = w // CW
    wr = w % CW
    return c * (C * 128 * CW) + (n // NP) * (128 * CW) + p * CW + wr


def _preprocess(edge_index):
    N = CFG["N_NODES"]
    C = CFG["CORES"]
    NP = N // C
    WIN = NP // 128
    AGC = CFG["AGC"]
    CW = WIN // AGC
    CBLK = C * 128 * CW  # rows per chunk tensor
    S = AGC
    assert WIN % AGC == 0
    assert CBLK <= 32768  # int16 gather indices

    src = np.asarray(edge_index[0], dtype=np.int64)
    dst = np.asarray(edge_index[1], dtype=np.int64)
    deg = np.bincount(dst, minlength=N).astype(np.float32) + 1.0

    row_of = _row_of_node(np.arange(N, dtype=np.int64), NP, WIN, C, CW)

    pr = Prep()
    pr.N, pr.C, pr.NP, pr.WIN = N, C, NP, WIN
    pr.AGC, pr.CW, pr.CBLK, pr.S = AGC, CW, CBLK, S

    # per-core, per-stream (== per-chunk) sorted message lists
    core_ld = [[None] * S for _ in range(C)]  # local dst per stream
    core_idx = [[None] * S for _ in range(C)]  # within-chunk table idx
    for i in range(C):
        m = (dst >= i * NP) & (dst < (i + 1) * NP)
        s_i = src[m]
        ld_i = dst[m] - i * NP
        order = np.argsort(ld_i, kind="stable")
        s_i, ld_i = s_i[order], ld_i[order]
        rows = row_of[s_i]
        cs = rows // CBLK
        for s in range(S):
            a = cs == s
            core_ld[i][s] = ld_i[a]
            core_idx[i][s] = rows[a] % CBLK

    # re-align all cores' streams at every RESYNC_G windows: within a group,
    # pad each core's segment to the max core's tile count. (G=WIN: only the
    # global per-stream tile count is equalized across cores.)
    G = CFG.get("RESYNC_G", 64)
    n_groups = (WIN + G - 1) // G
    for s in range(S):
        seg_tiles = np.zeros(n_groups, dtype=np.int64)
        for g in range(n_groups):
            lo, hi = g * G * 128, min((g + 1) * G, WIN) * 128
            for i in range(C):
                cnt = int(((core_ld[i][s] >= lo) & (core_ld[i][s] < hi)).sum())
                seg_tiles[g] = max(seg_tiles[g], (cnt + 127) // 128)
        for i in range(C):
            lds, ixs = [], []
            for g in range(n_groups):
                lo, hi = g * G * 128, min((g + 1) * G, WIN) * 128
                m = (core_ld[i][s] >= lo) & (core_ld[i][s] < hi)
                ld_g, ix_g = core_ld[i][s][m], core_idx[i][s][m]
                pad = int(seg_tiles[g]) * 128 - len(ld_g)
                lds.append(np.concatenate([ld_g, np.full(pad, SENT, np.int64)]))
                ixs.append(np.concatenate([ix_g, np.zeros(pad, np.int64)]))
            core_ld[i][s] = np.concatenate(lds)
            core_idx[i][s] = np.concatenate(ixs)
    T = [len(core_ld[0][s]) // 128 for s in range(S)]
    pr.T = T

    for i in range(C):
        for s in range(S):
            assert len(core_ld[i][s]) == T[s] * 128

    # union pair structure (w, stream, tile) across cores
    pair_set = set()
    for i in range(C):
        for s in range(S):
            L = core_ld[i][s].reshape(T[s], 128)
            for t in range(T[s]):
                real = L[t][L[t] != SENT]
                if len(real) == 0:
                    continue
                for w in range(int(real.min()) // 128, int(real.max()) // 128 + 1):
                    pair_set.add((w, s, t))
    for w in range(WIN):  # every window needs >=1 pair so psum gets reset
        if not any(p[0] == w for p in pair_set):
            pair_set.add((w, 0, 0))
    pairs = sorted(pair_set)
    pr.pairs = pairs
    pr.n_pairs = len(pairs)
    segs = [[] for _ in range(WIN)]
    for k, (w, s, t) in enumerate(pairs):
        segs[w].append(k)
    pr.segs = segs

    # per-core one-hot tiles [128, n_pairs, 128] fp8e4m3(0/1); the PE takes
    # fp8 stationary against bf16 moving directly, so no cast is needed
    pr.onehot = []
    pr.idx_wrapped = []
    pr.deg_staged = []
    FP8_ONE = 0x38
    for i in range(C):
        oh = np.zeros((128, pr.n_pairs, 128), dtype=np.uint8)
        for k, (w, s, t) in enumerate(pairs):
            ld_t = core_ld[i][s][t * 128 : (t + 1) * 128]
            slot = ld_t - 128 * w
            valid = (slot >= 0) & (slot < 128)
            rr = np.nonzero(valid)[0]
            oh[rr, k, slot[rr]] = FP8_ONE
        pr.onehot.append(oh.view(ml_dtypes.float8_e4m3fn))

        blocks = []
        for s in range(S):
            ix = core_idx[i][s].astype(np.int16)
            assert (core_idx[i][s] < CBLK).all() and (core_idx[i][s] >= 0).all()
            w16 = ix.reshape(-1, 16).T  # [16, T*8]
            blocks.append(np.tile(w16, (8, 1)))  # replicate to 128 partitions
        pr.idx_wrapped.append(
            np.ascontiguousarray(np.concatenate(blocks, axis=1))
        )

        dshard = deg[i * NP : (i + 1) * NP]
        pr.deg_staged.append(
            np.ascontiguousarray(dshard.reshape(WIN, 128).T.astype(np.float32))
        )

    return pr


# ------------------------------------------------------------------ bass ----


def _build(pr):
    import concourse.bass as bass
    import concourse.bacc as bacc
    import concourse.mybir as mybir
    import concourse.tile as tile
    from concourse._compat import get_trn_type

    dt = mybir.dt
    Alu = mybir.AluOpType
    F32, BF16, FP8, I16 = dt.float32, dt.bfloat16, dt.float8e4, dt.int16

    IN_C, OUT_C = CFG["IN_C"], CFG["OUT_C"]
    OC_PAD, CH, CHP = CFG["OC_PAD"], CFG["CH"], CFG["CHP"]
    N, C, NP, WIN = pr.N, pr.C, pr.NP, pr.WIN
    AGC, CW, CBLK, S = pr.AGC, pr.CW, pr.CBLK, pr.S
    CROWS = 128 * CW  # local rows per AG chunk
    T = pr.T
    NQ = min(S, 4)

    nc = bacc.Bacc(
        get_trn_type() or "TRN2",
        target_bir_lowering=False,
        debug=False,
        num_devices=C,
        num_swdge_queues=NQ,
    )

    xt_d = nc.dram_tensor("xt", [IN_C, NP], BF16, kind="ExternalInput")
    wt_d = nc.dram_tensor("wt", [IN_C, OUT_C], BF16, kind="ExternalInput")
    b_d = nc.dram_tensor("bias", [128, OUT_C], F32, kind="ExternalInput")
    deg_d = nc.dram_tensor("deg", [128, WIN], F32, kind="ExternalInput")
    idx_d = nc.dram_tensor(
        "idx", [128, sum(T) * 8], I16, kind="ExternalInput"
    )
    oh_d = nc.dram_tensor("oh", [128, pr.n_pairs, 128], FP8, kind="ExternalInput")
    out_d = nc.dram_tensor("out", [128, WIN, OUT_C], F32, kind="ExternalOutput")

    rg = [list(range(C))]

    with tile.TileContext(nc) as tc:
        with (
            tc.tile_pool(name="const", bufs=1) as const,
            tc.tile_pool(name="dram", bufs=1, space="DRAM") as dram,
            tc.tile_pool(name="psum_y", bufs=2, space="PSUM") as psum_y,
            tc.tile_pool(name="psum_w", bufs=CFG["PSUM_BUFS"], space="PSUM") as psum_w,
            tc.tile_pool(name="msg0", bufs=CFG["MSG_BUFS"]) as msg0_pool,
            tc.tile_pool(name="msg1", bufs=CFG["MSG_BUFS"]) as msg1_pool,
            tc.tile_pool(name="msg2", bufs=CFG["MSG_BUFS"]) as msg2_pool,
            tc.tile_pool(name="msg3", bufs=CFG["MSG_BUFS"]) as msg3_pool,
            tc.tile_pool(name="ohp", bufs=CFG["OH_BUFS"]) as oh_pool,
            tc.tile_pool(name="tmp", bufs=4) as tmp_pool,
        ):
            cc1_in = dram.tile([NP, OC_PAD], BF16)
            cc2_in = dram.tile([NP, OC_PAD], BF16)
            cc1_outs = [
                dram.tile(
                    [CBLK, OC_PAD], BF16, addr_space="Shared", name=f"cc1o{c}"
                )
                for c in range(AGC)
            ]
            cc2_outs = [
                dram.tile(
                    [CBLK, OC_PAD], BF16, addr_space="Shared", name=f"cc2o{c}"
                )
                for c in range(AGC)
            ]

            idx_sb = const.tile([128, sum(T) * 8], I16)
            nc.sync.dma_start(idx_sb[:], idx_d[:])
            wt_sb = const.tile([IN_C, OUT_C], BF16)
            nc.sync.dma_start(wt_sb[:], wt_d[:])
            b_sb = const.tile([128, OUT_C], F32)
            nc.sync.dma_start(b_sb[:], b_d[:])
            deg_sb = const.tile([128, WIN], F32)
            nc.sync.dma_start(deg_sb[:], deg_d[:])
            xt_sb = const.tile([IN_C, NP], BF16)
            nc.sync.dma_start(xt_sb[:], xt_d[:])

            deginv = const.tile([128, WIN], F32)
            nc.vector.reciprocal(deginv[:], deg_sb[:])
            dinv = const.tile([128, WIN], F32)
            nc.scalar.activation(
                dinv[:], deginv[:], mybir.ActivationFunctionType.Sqrt
            )

            z0f = const.tile([128, WIN, OUT_C], F32)
            z1f = const.tile([128, WIN, OUT_C], F32)
            outst = const.tile([128, WIN, OUT_C], F32)
            zpad1 = const.tile([128, WIN, OC_PAD], BF16)
            zpad2 = const.tile([128, WIN, OC_PAD], BF16)
            nc.vector.memset(zpad1[:], 0.0)
            nc.vector.memset(zpad2[:], 0.0)

            STAGE = CFG["STAGE"]

            def emit_ag(cc_in, cc_outs, c, zpad):
                nc.sync.dma_start(
                    cc_in[c * CROWS : (c + 1) * CROWS, :],
                    zpad[:, c * CW : (c + 1) * CW, :],
                )
                nc.gpsimd.collective_compute(
                    "AllGather",
                    Alu.bypass,
                    replica_groups=rg,
                    ins=[cc_in[c * CROWS : (c + 1) * CROWS, :].opt()],
                    outs=[cc_outs[c][:].opt()],
                )

            # ---- projection: z0 = dinv * (x @ W^T), staged [p, w, ch] ----
            for r in range(WIN):
                py = psum_y.tile([128, OUT_C], F32)
                nc.tensor.matmul(
                    py[:],
                    xt_sb[:, r * 128 : (r + 1) * 128],
                    wt_sb[:],
                    start=True,
                    stop=True,
                )
                if CFG["ACT_EVAC"]:
                    nc.scalar.mul(z0f[:, r, :], py[:], dinv[:, r : r + 1])
                    nc.scalar.copy(zpad1[:, r, 0:OUT_C], z0f[:, r, :])
                else:
                    nc.vector.tensor_scalar(
                        z0f[:, r, :], py[:], dinv[:, r : r + 1], None, Alu.mult
                    )
                    nc.vector.tensor_copy(zpad1[:, r, 0:OUT_C], z0f[:, r, :])
                if STAGE >= 2 and (r + 1) % CW == 0:
                    emit_ag(cc1_in, cc1_outs, r // CW, zpad1)

            calls = [(T[s] + CH - 1) // CH for s in range(S)]
            n_oh_chunks = (pr.n_pairs + CHP - 1) // CHP
            colbase = np.concatenate([[0], np.cumsum(np.array(T) * 8)])
            msg_pools = [msg0_pool, msg1_pool, msg2_pool, msg3_pool][:S]
            gq_counter = [0]

            def run_hop(cc_outs, evac, do_mm=True, after_window=None):
                tabs = [cc_outs[s][:] for s in range(S)]
                msg_tiles = [{} for _ in range(S)]
                oh_tiles = {}
                next_call = [0] * S
                next_oh = [0]

                def emit_gather(s):
                    c = next_call[s]
                    ntiles = min(CH, T[s] - c * CH)
                    ni = ntiles * 128
                    t = msg_pools[s].tile([128, CH, OC_PAD], BF16, tag=f"msg{s}")
                    sl = slice(
                        int(colbase[s]) + c * CH * 8,
                        int(colbase[s]) + c * CH * 8 + ntiles * 8,
                    )
                    nc.gpsimd.dma_gather(
                        t[:, 0:ntiles, :],
                        tabs[s],
                        idx_sb[:, sl],
                        ni,
                        ni,
                        OC_PAD,
                        single_packet=(ni <= 1024),
                        queue_num=(gq_counter[0] if CFG.get("QRR") else s) % NQ,
                    )
                    gq_counter[0] += 1
                    msg_tiles[s][c] = t
                    next_call[s] = c + 1

                def emit_oh():
                    k = next_oh[0]
                    npair = min(CHP, pr.n_pairs - k * CHP)
                    t = oh_pool.tile([128, CHP, 128], FP8, tag="oh")
                    nc.sync.dma_start(
                        out=t[:, 0:npair, :],
                        in_=oh_d[:, k * CHP : k * CHP + npair, :],
                    )
                    oh_tiles[k] = t
                    next_oh[0] = k + 1

                for w in range(WIN):
                    seg = pr.segs[w]
                    # make sure resources (plus prefetch) exist
                    for pk in seg:
                        _, s, t = pr.pairs[pk]
                        while next_call[s] <= min(
                            t // CH + CFG.get("PF", 1), calls[s] - 1
                        ):
                            emit_gather(s)
                        while next_oh[0] <= min(pk // CHP + 1, n_oh_chunks - 1):
                            emit_oh()
                    if not do_mm:
                        continue
                    pw = psum_w.tile([128, OUT_C], F32)
                    for j, pk in enumerate(seg):
                        _, s, t = pr.pairs[pk]
                        oh_ap = oh_tiles[pk // CHP][:, pk % CHP, :]
                        msg_ap = msg_tiles[s][t // CH][:, t % CH, 0:OUT_C]
                        nc.tensor.matmul(
                            pw[:],
                            oh_ap,
                            msg_ap,
                            start=(j == 0),
                            stop=(j == len(seg) - 1),
                        )
                    evac(w, pw)
                    if after_window is not None:
                        after_window(w)

            # ---- hop 1:  z1 = (psum + z0) / deg ----
            def evac1(w, pw):
                tmp = tmp_pool.tile([128, OUT_C], F32, tag="tmp")
                nc.vector.tensor_add(tmp[:], pw[:], z0f[:, w, :])
                if CFG["ACT_EVAC"]:
                    nc.scalar.mul(z1f[:, w, :], tmp[:], deginv[:, w : w + 1])
                    nc.scalar.mul(
                        zpad2[:, w, 0:OUT_C], tmp[:], deginv[:, w : w + 1]
                    )
                else:
                    nc.vector.tensor_scalar(
                        z1f[:, w, :], tmp[:], deginv[:, w : w + 1], None, Alu.mult
                    )
                    nc.vector.tensor_copy(zpad2[:, w, 0:OUT_C], z1f[:, w, :])

            def after1(w):
                if STAGE >= 5 and (w + 1) % CW == 0:
                    emit_ag(cc2_in, cc2_outs, w // CW, zpad2)

            if STAGE >= 3:
                run_hop(cc1_outs, evac1, do_mm=STAGE >= 4, after_window=after1)

            # ---- hop 2:  out = dinv * (psum + z1) + b ----
            def evac2(w, pw):
                tmp = tmp_pool.tile([128, OUT_C], F32, tag="tmp")
                tmp2 = tmp_pool.tile([128, OUT_C], F32, tag="tmp2")
                nc.vector.tensor_add(tmp[:], pw[:], z1f[:, w, :])
                if CFG["ACT_EVAC"]:
                    nc.scalar.mul(tmp2[:], tmp[:], dinv[:, w : w + 1])
                else:
                    nc.vector.tensor_scalar(
                        tmp2[:], tmp[:], dinv[:, w : w + 1], None, Alu.mult
                    )
                nc.vector.tensor_add(outst[:, w, :], tmp2[:], b_sb[:])

            def after2(w):
                if (w + 1) % CW == 0:
                    c = w // CW
                    nc.sync.dma_start(
                        out_d[:, c * CW : (c + 1) * CW, :],
                        outst[:, c * CW : (c + 1) * CW, :],
                    )

            if STAGE >= 6:
                run_hop(cc2_outs, evac2, after_window=after2)
            else:
                src_final = {1: z0f, 2: z0f, 3: z0f, 4: z1f, 5: z1f}[STAGE]
                nc.sync.dma_start(out_d[:], src_final[:])

    nc.compile()
    return nc


def _make_in_maps(pr, x, W, b):
    C, NP, WIN = pr.C, pr.NP, pr.WIN
    x = np.asarray(x, dtype=np.float32)
    W = np.asarray(W, dtype=np.float32)
    b = np.asarray(b, dtype=np.float32)
    wt = np.ascontiguousarray(W.T.astype(ml_dtypes.bfloat16))
    b_rep = np.ascontiguousarray(np.broadcast_to(b, (128, len(b))))
    in_maps = []
    for i in range(C):
        xt = np.ascontiguousarray(
            x[i * NP : (i + 1) * NP].T.astype(ml_dtypes.bfloat16)
        )
        in_maps.append(
            dict(
                xt=xt,
                wt=wt,
                bias=b_rep,
                deg=pr.deg_staged[i],
                idx=pr.idx_wrapped[i],
                oh=pr.onehot[i],
            )
        )
    return in_maps


def _unpermute(o, pr):
    # device layout is [p, w, ch]; node order is w*128+p
    return (
        o.reshape(128, pr.WIN, o.shape[-1])
        .transpose(1, 0, 2)
        .reshape(pr.NP, o.shape[-1])
    )


def kernel(x, edge_index, W, b):
    pr = _preprocess(edge_index)
    nc = _build(pr)
    in_maps = _make_in_maps(pr, x, W, b)

    from concourse import bass_utils

    res = bass_utils.run_bass_kernel_spmd(
        nc, in_maps, core_ids=list(range(pr.C))
    )
    shards = [_unpermute(res.results[i]["out"], pr) for i in range(pr.C)]
    return np.ascontiguousarray(np.concatenate(shards, axis=0))


# revision 47
# speedup vs baseline: 1.2548x; 1.0475x over previous
"""SGC (2-hop simple graph convolution) Trainium2 kernel, 8-core SPMD.

out = S S x W^T + b,  S = D^{-1/2} (A + I) D^{-1/2}   (D = in-degree + 1)

Strategy:
  * project first: y = x @ W^T (64 ch), exact by associativity
  * factor norms:  S z = dinv * (A+I) (dinv * z)  -> per-node scalings only,
    messages are unweighted; self loop handled as a local add
  * per core: own 1/8 of destination nodes; edges partitioned by dst
  * z tables are AllGather'ed in AGC chunks (chunk-major node permutation so
    every chunk is a contiguous Shared tensor); each chunk is its own gather
    "stream" pinned to its own SWDGE queue, so chunk-s gathers start as soon
    as chunk-s AG lands and descriptor generation runs 4-wide on gpsimd
  * gather sources with gpsimd dma_gather from the bf16 chunk tables
    (rows padded to 128 ch = 256 B to satisfy the elem%256 constraint);
    ni<=1024 per call enables single_packet
  * scatter-adds via PE matmul: 128-message tiles x host-built 0/1 one-hot
    stationary tiles (fp8, streamed on the sync HWDGE queue; the PE takes fp8
    stationary against bf16 moving directly), accumulated in PSUM per
    128-destination window; out-of-window slots give all-zero rows so stream
    tiles may straddle windows with no padding
  * node numbering inside tables is chunk-major permuted; the output staging
    keeps the [p, w] permutation and the host un-permutes at the end
"""

import sys

sys.path.insert(0, "/opt/trn_rl_repo")

import numpy as np
import ml_dtypes

# ---------------- problem constants (overridden by tests for small runs) ----
CFG = dict(
    N_NODES=65536,
    N_EDGES=655360,
    IN_C=128,
    OUT_C=64,
    CORES=8,
    CH=8,  # gather tiles (128 msgs each) per dma_gather call; ni<=1024 -> single_packet
    CHP=32,  # one-hot pairs per DMA chunk
    OC_PAD=128,  # bf16 channels per gather-table row (256 B)
    MSG_BUFS=8,  # per-stream message buffers
    OH_BUFS=3,
    PF=4,  # gather chunks prefetched ahead of demand
    PSUM_BUFS=6,
    RESYNC_G=64,  # no intra-stream alignment: fewest gather tiles wins
    ACT_EVAC=1,
    STAGE=6,  # debug: 1 proj, 2 +ag1, 3 +gather/oh, 4 +hop1 mm, 5 +ag2, 6 full
    AGC=4,  # AllGather chunks == gather streams == SWDGE queues
    QRR=0,  # 1: round-robin gather queue assignment; 0: queue = stream id
    AGD=2,  # windows to delay non-final AG2 triggers (lets staging finish)
    SCQ=1,  # 1: AG staging + out chunks on scalar HWDGE queue
)

SENT = 1 << 20  # sentinel "dst" for pad rows -> all-zero one-hot everywhere


class Prep:
    pass


def _row_of_node(n, NP, WIN, C, CW):
    # node n -> chunk-major gather-table row so each AG chunk is contiguous:
    # row = c*(C*128*CW) + core*(128*CW) + p*CW + w%CW   (p = r%128, w = r//128)
    r = n % NP
    p = r % 128
    w = r // 128
    c = w // CW
    wr = w % CW
    return c * (C * 128 * CW) + (n // NP) * (128 * CW) + p * CW + wr


def _preprocess(edge_index):
    N = CFG["N_NODES"]
    C = CFG["CORES"]
    NP = N // C
    WIN = NP // 128
    AGC = CFG["AGC"]
    CW = WIN // AGC
    CBLK = C * 128 * CW  # rows per chunk tensor
    S = AGC
    assert WIN % AGC == 0
    assert CBLK <= 32768  # int16 gather indices

    src = np.asarray(edge_index[0], dtype=np.int64)
    dst = np.asarray(edge_index[1], dtype=np.int64)
    deg = np.bincount(dst, minlength=N).astype(np.float32) + 1.0

    row_of = _row_of_node(np.arange(N, dtype=np.int64), NP, WIN, C, CW)

    pr = Prep()
    pr.N, pr.C, pr.NP, pr.WIN = N, C, NP, WIN
    pr.AGC, pr.CW, pr.CBLK, pr.S = AGC, CW, CBLK, S

    # per-core, per-stream (== per-chunk) sorted message lists
    core_ld = [[None] * S for _ in range(C)]  # local dst per stream
    core_idx = [[None] * S for _ in range(C)]  # within-chunk table idx
    for i in range(C):
        m = (dst >= i * NP) & (dst < (i + 1) * NP)
        s_i = src[m]
        ld_i = dst[m] - i * NP
        order = np.argsort(ld_i, kind="stable")
        s_i, ld_i = s_i[order], ld_i[order]
        rows = row_of[s_i]
        cs = rows // CBLK
        for s in range(S):
            a = cs == s
            core_ld[i][s] = ld_i[a]
            core_idx[i][s] = rows[a] % CBLK

    # re-align all cores' streams at every RESYNC_G windows: within a group,
    # pad each core's segment to the max core's tile count. (G=WIN: only the
    # global per-stream tile count is equalized across cores.)
    G = CFG.get("RESYNC_G", 64)
    n_groups = (WIN + G - 1) // G
    for s in range(S):
        seg_tiles = np.zeros(n_groups, dtype=np.int64)
        for g in range(n_groups):
            lo, hi = g * G * 128, min((g + 1) * G, WIN) * 128
            for i in range(C):
                cnt = int(((core_ld[i][s] >= lo) & (core_ld[i][s] < hi)).sum())
                seg_tiles[g] = max(seg_tiles[g], (cnt + 127) // 128)
        for i in range(C):
            lds, ixs = [], []
            for g in range(n_groups):
                lo, hi = g * G * 128, min((g + 1) * G, WIN) * 128
                m = (core_ld[i][s] >= lo) & (core_ld[i][s] < hi)
                ld_g, ix_g = core_ld[i][s][m], core_idx[i][s][m]
                pad = int(seg_tiles[g]) * 128 - len(ld_g)
                lds.append(np.concatenate([ld_g, np.full(pad, SENT, np.int64)]))
                ixs.append(np.concatenate([ix_g, np.zeros(pad, np.int64)]))
            core_ld[i][s] = np.concatenate(lds)
            core_idx[i][s] = np.concatenate(ixs)
    T = [len(core_ld[0][s]) // 128 for s in range(S)]
    pr.T = T

    for i in range(C):
        for s in range(S):
            assert len(core_ld[i][s]) == T[s] * 128

    # union pair structure (w, stream, tile) across cores
    pair_set = set()
    for i in range(C):
        for s in range(S):
            L = core_ld[i][s].reshape(T[s], 128)
            for t in range(T[s]):
                real = L[t][L[t] != SENT]
                if len(real) == 0:
                    continue
                for w in range(int(real.min()) // 128, int(real.max()) // 128 + 1):
                    pair_set.add((w, s, t))
    for w in range(WIN):  # every window needs >=1 pair so psum gets reset
        if not any(p[0] == w for p in pair_set):
            pair_set.add((w, 0, 0))
    pairs = sorted(pair_set)
    pr.pairs = pairs
    pr.n_pairs = len(pairs)
    segs = [[] for _ in range(WIN)]
    for k, (w, s, t) in enumerate(pairs):
        segs[w].append(k)
    pr.segs = segs

    # per-core one-hot tiles [128, n_pairs, 128] fp8e4m3(0/1); the PE takes
    # fp8 stationary against bf16 moving directly, so no cast is needed
    pr.onehot = []
    pr.idx_wrapped = []
    pr.deg_staged = []
    FP8_ONE = 0x38
    for i in range(C):
        oh = np.zeros((128, pr.n_pairs, 128), dtype=np.uint8)
        for k, (w, s, t) in enumerate(pairs):
            ld_t = core_ld[i][s][t * 128 : (t + 1) * 128]
            slot = ld_t - 128 * w
            valid = (slot >= 0) & (slot < 128)
            rr = np.nonzero(valid)[0]
            oh[rr, k, slot[rr]] = FP8_ONE
        pr.onehot.append(oh.view(ml_dtypes.float8_e4m3fn))

        blocks = []
        for s in range(S):
            ix = core_idx[i][s].astype(np.int16)
            assert (core_idx[i][s] < CBLK).all() and (core_idx[i][s] >= 0).all()
            w16 = ix.reshape(-1, 16).T  # [16, T*8]
            blocks.append(np.tile(w16, (8, 1)))  # replicate to 128 partitions
        pr.idx_wrapped.append(
            np.ascontiguousarray(np.concatenate(blocks, axis=1))
        )

        dshard = deg[i * NP : (i + 1) * NP]
        pr.deg_staged.append(
            np.ascontiguousarray(dshard.reshape(WIN, 128).T.astype(np.float32))
        )

    return pr


# ------------------------------------------------------------------ bass ----


def _build(pr):
    import concourse.bass as bass
    import concourse.bacc as bacc
    import concourse.mybir as mybir
    import concourse.tile as tile
    from concourse._compat import get_trn_type

    dt = mybir.dt
    Alu = mybir.AluOpType
    F32, BF16, FP8, I16 = dt.float32, dt.bfloat16, dt.float8e4, dt.int16

    IN_C, OUT_C = CFG["IN_C"], CFG["OUT_C"]
    OC_PAD, CH, CHP = CFG["OC_PAD"], CFG["CH"], CFG["CHP"]
    N, C, NP, WIN = pr.N, pr.C, pr.NP, pr.WIN
    AGC, CW, CBLK, S = pr.AGC, pr.CW, pr.CBLK, pr.S
    CROWS = 128 * CW  # local rows per AG chunk
    T = pr.T
    NQ = min(S, 4)

    nc = bacc.Bacc(
        get_trn_type() or "TRN2",
        target_bir_lowering=False,
        debug=False,
        num_devices=C,
        num_swdge_queues=NQ,
    )

    xt_d = nc.dram_tensor("xt", [IN_C, NP], BF16, kind="ExternalInput")
    wt_d = nc.dram_tensor("wt", [IN_C, OUT_C], BF16, kind="ExternalInput")
    b_d = nc.dram_tensor("bias", [128, OUT_C], F32, kind="ExternalInput")
    deg_d = nc.dram_tensor("deg", [128, WIN], F32, kind="ExternalInput")
    idx_d = nc.dram_tensor(
        "idx", [128, sum(T) * 8], I16, kind="ExternalInput"
    )
    oh_d = nc.dram_tensor("oh", [128, pr.n_pairs, 128], FP8, kind="ExternalInput")
    out_d = nc.dram_tensor("out", [128, WIN, OUT_C], F32, kind="ExternalOutput")

    rg = [list(range(C))]

    with tile.TileContext(nc) as tc:
        with (
            tc.tile_pool(name="const", bufs=1) as const,
            tc.tile_pool(name="dram", bufs=1, space="DRAM") as dram,
            tc.tile_pool(name="psum_y", bufs=2, space="PSUM") as psum_y,
            tc.tile_pool(name="psum_w", bufs=CFG["PSUM_BUFS"], space="PSUM") as psum_w,
            tc.tile_pool(name="msg0", bufs=CFG["MSG_BUFS"]) as msg0_pool,
            tc.tile_pool(name="msg1", bufs=CFG["MSG_BUFS"]) as msg1_pool,
            tc.tile_pool(name="msg2", bufs=CFG["MSG_BUFS"]) as msg2_pool,
            tc.tile_pool(name="msg3", bufs=CFG["MSG_BUFS"]) as msg3_pool,
            tc.tile_pool(name="ohp", bufs=CFG["OH_BUFS"]) as oh_pool,
            tc.tile_pool(name="tmp", bufs=4) as tmp_pool,
        ):
            cc1_in = dram.tile([NP, OC_PAD], BF16)
            cc2_in = dram.tile([NP, OC_PAD], BF16)
            cc1_outs = [
                dram.tile(
                    [CBLK, OC_PAD], BF16, addr_space="Shared", name=f"cc1o{c}"
                )
                for c in range(AGC)
            ]
            cc2_outs = [
                dram.tile(
                    [CBLK, OC_PAD], BF16, addr_space="Shared", name=f"cc2o{c}"
                )
                for c in range(AGC)
            ]

            idx_sb = const.tile([128, sum(T) * 8], I16)
            nc.sync.dma_start(idx_sb[:], idx_d[:])
            wt_sb = const.tile([IN_C, OUT_C], BF16)
            nc.sync.dma_start(wt_sb[:], wt_d[:])
            b_sb = const.tile([128, OUT_C], F32)
            nc.sync.dma_start(b_sb[:], b_d[:])
            deg_sb = const.tile([128, WIN], F32)
            nc.sync.dma_start(deg_sb[:], deg_d[:])
            xt_sb = const.tile([IN_C, NP], BF16)
            nc.sync.dma_start(xt_sb[:], xt_d[:])

            deginv = const.tile([128, WIN], F32)
            nc.vector.reciprocal(deginv[:], deg_sb[:])
            dinv = const.tile([128, WIN], F32)
            nc.scalar.activation(
                dinv[:], deginv[:], mybir.ActivationFunctionType.Sqrt
            )

            z0f = const.tile([128, WIN, OUT_C], F32)
            z1f = const.tile([128, WIN, OUT_C], F32)
            outst = const.tile([128, WIN, OUT_C], F32)
            zpad1 = const.tile([128, WIN, OC_PAD], BF16)
            zpad2 = const.tile([128, WIN, OC_PAD], BF16)
            nc.vector.memset(zpad1[:], 0.0)
            nc.vector.memset(zpad2[:], 0.0)

            STAGE = CFG["STAGE"]

            stage_eng = nc.scalar if CFG.get("SCQ") else nc.sync

            def emit_ag(cc_in, cc_outs, c, zpad):
                stage_eng.dma_start(
                    cc_in[c * CROWS : (c + 1) * CROWS, :],
                    zpad[:, c * CW : (c + 1) * CW, :],
                )
                nc.gpsimd.collective_compute(
                    "AllGather",
                    Alu.bypass,
                    replica_groups=rg,
                    ins=[cc_in[c * CROWS : (c + 1) * CROWS, :].opt()],
                    outs=[cc_outs[c][:].opt()],
                )

            # ---- projection: z0 = dinv * (x @ W^T), staged [p, w, ch] ----
            for r in range(WIN):
                py = psum_y.tile([128, OUT_C], F32)
                nc.tensor.matmul(
                    py[:],
                    xt_sb[:, r * 128 : (r + 1) * 128],
                    wt_sb[:],
                    start=True,
                    stop=True,
                )
                if CFG["ACT_EVAC"]:
                    nc.scalar.mul(z0f[:, r, :], py[:], dinv[:, r : r + 1])
                    nc.scalar.copy(zpad1[:, r, 0:OUT_C], z0f[:, r, :])
                else:
                    nc.vector.tensor_scalar(
                        z0f[:, r, :], py[:], dinv[:, r : r + 1], None, Alu.mult
                    )
                    nc.vector.tensor_copy(zpad1[:, r, 0:OUT_C], z0f[:, r, :])
                if STAGE >= 2 and (r + 1) % CW == 0:
                    emit_ag(cc1_in, cc1_outs, r // CW, zpad1)

            calls = [(T[s] + CH - 1) // CH for s in range(S)]
            n_oh_chunks = (pr.n_pairs + CHP - 1) // CHP
            colbase = np.concatenate([[0], np.cumsum(np.array(T) * 8)])
            msg_pools = [msg0_pool, msg1_pool, msg2_pool, msg3_pool][:S]
            gq_counter = [0]

            def run_hop(cc_outs, evac, do_mm=True, after_window=None):
                tabs = [cc_outs[s][:] for s in range(S)]
                msg_tiles = [{} for _ in range(S)]
                oh_tiles = {}
                next_call = [0] * S
                next_oh = [0]

                def emit_gather(s):
                    c = next_call[s]
                    ntiles = min(CH, T[s] - c * CH)
                    ni = ntiles * 128
                    t = msg_pools[s].tile([128, CH, OC_PAD], BF16, tag=f"msg{s}")
                    sl = slice(
                        int(colbase[s]) + c * CH * 8,
                        int(colbase[s]) + c * CH * 8 + ntiles * 8,
                    )
                    nc.gpsimd.dma_gather(
                        t[:, 0:ntiles, :],
                        tabs[s],
                        idx_sb[:, sl],
                        ni,
                        ni,
                        OC_PAD,
                        single_packet=(ni <= 1024),
                        queue_num=(gq_counter[0] if CFG.get("QRR") else s) % NQ,
                    )
                    gq_counter[0] += 1
                    msg_tiles[s][c] = t
                    next_call[s] = c + 1

                def emit_oh():
                    k = next_oh[0]
                    npair = min(CHP, pr.n_pairs - k * CHP)
                    t = oh_pool.tile([128, CHP, 128], FP8, tag="oh")
                    nc.sync.dma_start(
                        out=t[:, 0:npair, :],
                        in_=oh_d[:, k * CHP : k * CHP + npair, :],
                    )
                    oh_tiles[k] = t
                    next_oh[0] = k + 1

                for w in range(WIN):
                    seg = pr.segs[w]
                    # make sure resources (plus prefetch) exist
                    for pk in seg:
                        _, s, t = pr.pairs[pk]
                        while next_call[s] <= min(
                            t // CH + CFG.get("PF", 1), calls[s] - 1
                        ):
                            emit_gather(s)
                        while next_oh[0] <= min(pk // CHP + 1, n_oh_chunks - 1):
                            emit_oh()
                    if not do_mm:
                        continue
                    pw = psum_w.tile([128, OUT_C], F32)
                    for j, pk in enumerate(seg):
                        _, s, t = pr.pairs[pk]
                        oh_ap = oh_tiles[pk // CHP][:, pk % CHP, :]
                        msg_ap = msg_tiles[s][t // CH][:, t % CH, 0:OUT_C]
                        nc.tensor.matmul(
                            pw[:],
                            oh_ap,
                            msg_ap,
                            start=(j == 0),
                            stop=(j == len(seg) - 1),
                        )
                    evac(w, pw)
                    if after_window is not None:
                        after_window(w)

            # ---- hop 1:  z1 = (psum + z0) / deg ----
            def evac1(w, pw):
                tmp = tmp_pool.tile([128, OUT_C], F32, tag="tmp")
                nc.vector.tensor_add(tmp[:], pw[:], z0f[:, w, :])
                if CFG["ACT_EVAC"]:
                    nc.scalar.mul(z1f[:, w, :], tmp[:], deginv[:, w : w + 1])
                    nc.scalar.mul(
                        zpad2[:, w, 0:OUT_C], tmp[:], deginv[:, w : w + 1]
                    )
                else:
                    nc.vector.tensor_scalar(
                        z1f[:, w, :], tmp[:], deginv[:, w : w + 1], None, Alu.mult
                    )
                    nc.vector.tensor_copy(zpad2[:, w, 0:OUT_C], z1f[:, w, :])

            agd = CFG.get("AGD", 0)
            ag2_fire = {
                min((c + 1) * CW - 1 + (agd if c < AGC - 1 else 0), WIN - 1): c
                for c in range(AGC)
            }

            def after1(w):
                if STAGE >= 5 and w in ag2_fire:
                    emit_ag(cc2_in, cc2_outs, ag2_fire[w], zpad2)

            if STAGE >= 3:
                run_hop(cc1_outs, evac1, do_mm=STAGE >= 4, after_window=after1)

            # ---- hop 2:  out = dinv * (psum + z1) + b ----
            def evac2(w, pw):
                tmp = tmp_pool.tile([128, OUT_C], F32, tag="tmp")
                tmp2 = tmp_pool.tile([128, OUT_C], F32, tag="tmp2")
                nc.vector.tensor_add(tmp[:], pw[:], z1f[:, w, :])
                if CFG["ACT_EVAC"]:
                    nc.scalar.mul(tmp2[:], tmp[:], dinv[:, w : w + 1])
                else:
                    nc.vector.tensor_scalar(
                        tmp2[:], tmp[:], dinv[:, w : w + 1], None, Alu.mult
                    )
                nc.vector.tensor_add(outst[:, w, :], tmp2[:], b_sb[:])

            def after2(w):
                if (w + 1) % CW == 0:
                    c = w // CW
                    stage_eng.dma_start(
                        out_d[:, c * CW : (c + 1) * CW, :],
                        outst[:, c * CW : (c + 1) * CW, :],
                    )

            if STAGE >= 6:
                run_hop(cc2_outs, evac2, after_window=after2)
            else:
                src_final = {1: z0f, 2: z0f, 3: z0f, 4: z1f, 5: z1f}[STAGE]
                nc.sync.dma_start(out_d[:], src_final[:])

    nc.compile()
    return nc


def _make_in_maps(pr, x, W, b):
    C, NP, WIN = pr.C, pr.NP, pr.WIN
    x = np.asarray(x, dtype=np.float32)
    W = np.asarray(W, dtype=np.float32)
    b = np.asarray(b, dtype=np.float32)
    wt = np.ascontiguousarray(W.T.astype(ml_dtypes.bfloat16))
    b_rep = np.ascontiguousarray(np.broadcast_to(b, (128, len(b))))
    in_maps = []
    for i in range(C):
        xt = np.ascontiguousarray(
            x[i * NP : (i + 1) * NP].T.astype(ml_dtypes.bfloat16)
        )
        in_maps.append(
            dict(
                xt=xt,
                wt=wt,
                bias=b_rep,
                deg=pr.deg_staged[i],
                idx=pr.idx_wrapped[i],
                oh=pr.onehot[i],
            )
        )
    return in_maps


def _unpermute(o, pr):
    # device layout is [p, w, ch]; node order is w*128+p
    return (
        o.reshape(128, pr.WIN, o.shape[-1])
        .transpose(1, 0, 2)
        .reshape(pr.NP, o.shape[-1])
    )


def kernel(x, edge_index, W, b):
    pr = _preprocess(edge_index)
    nc = _build(pr)
    in_maps = _make_in_maps(pr, x, W, b)

    from concourse import bass_utils

    res = bass_utils.run_bass_kernel_spmd(
        nc, in_maps, core_ids=list(range(pr.C))
    )
    shards = [_unpermute(res.results[i]["out"], pr) for i in range(pr.C)]
    return np.ascontiguousarray(np.concatenate(shards, axis=0))


# revision 48
# speedup vs baseline: 1.2589x; 1.0033x over previous
"""SGC (2-hop simple graph convolution) Trainium2 kernel, 8-core SPMD.

out = S S x W^T + b,  S = D^{-1/2} (A + I) D^{-1/2}   (D = in-degree + 1)

Strategy:
  * project first: y = x @ W^T (64 ch), exact by associativity
  * factor norms:  S z = dinv * (A+I) (dinv * z)  -> per-node scalings only,
    messages are unweighted; self loop handled as a local add
  * per core: own 1/8 of destination nodes; edges partitioned by dst
  * z tables are AllGather'ed in AGC chunks (chunk-major node permutation so
    every chunk is a contiguous Shared tensor); each chunk is its own gather
    "stream" pinned to its own SWDGE queue, so chunk-s gathers start as soon
    as chunk-s AG lands and descriptor generation runs 4-wide on gpsimd
  * gather sources with gpsimd dma_gather from the bf16 chunk tables
    (rows padded to 128 ch = 256 B to satisfy the elem%256 constraint);
    ni<=1024 per call enables single_packet
  * scatter-adds via PE matmul: 128-message tiles x host-built 0/1 one-hot
    stationary tiles (fp8, streamed on the sync HWDGE queue; the PE takes fp8
    stationary against bf16 moving directly), accumulated in PSUM per
    128-destination window; out-of-window slots give all-zero rows so stream
    tiles may straddle windows with no padding
  * node numbering inside tables is chunk-major permuted; the output staging
    keeps the [p, w] permutation and the host un-permutes at the end
"""

import sys

sys.path.insert(0, "/opt/trn_rl_repo")

import numpy as np
import ml_dtypes

# ---------------- problem constants (overridden by tests for small runs) ----
CFG = dict(
    N_NODES=65536,
    N_EDGES=655360,
    IN_C=128,
    OUT_C=64,
    CORES=8,
    CH=8,  # gather tiles (128 msgs each) per dma_gather call; ni<=1024 -> single_packet
    CHP=32,  # one-hot pairs per DMA chunk
    OC_PAD=128,  # bf16 channels per gather-table row (256 B)
    MSG_BUFS=8,  # per-stream message buffers
    OH_BUFS=3,
    PF=4,  # gather chunks prefetched ahead of demand
    PSUM_BUFS=6,
    RESYNC_G=64,  # no intra-stream alignment: fewest gather tiles wins
    ACT_EVAC=1,
    STAGE=6,  # debug: 1 proj, 2 +ag1, 3 +gather/oh, 4 +hop1 mm, 5 +ag2, 6 full
    AGC=4,  # AllGather chunks == gather streams == SWDGE queues
    QRR=1,  # 1: round-robin gather queue assignment; 0: queue = stream id
    AGD=2,  # windows to delay non-final AG2 triggers (lets staging finish)
    SCQ=1,  # 1: AG staging + out chunks on scalar HWDGE queue
)

SENT = 1 << 20  # sentinel "dst" for pad rows -> all-zero one-hot everywhere


class Prep:
    pass


def _row_of_node(n, NP, WIN, C, CW):
    # node n -> chunk-major gather-table row so each AG chunk is contiguous:
    # row = c*(C*128*CW) + core*(128*CW) + p*CW + w%CW   (p = r%128, w = r//128)
    r = n % NP
    p = r % 128
    w = r // 128
    c = w // CW
    wr = w % CW
    return c * (C * 128 * CW) + (n // NP) * (128 * CW) + p * CW + wr


def _preprocess(edge_index):
    N = CFG["N_NODES"]
    C = CFG["CORES"]
    NP = N // C
    WIN = NP // 128
    AGC = CFG["AGC"]
    CW = WIN // AGC
    CBLK = C * 128 * CW  # rows per chunk tensor
    S = AGC
    assert WIN % AGC == 0
    assert CBLK <= 32768  # int16 gather indices

    src = np.asarray(edge_index[0], dtype=np.int64)
    dst = np.asarray(edge_index[1], dtype=np.int64)
    deg = np.bincount(dst, minlength=N).astype(np.float32) + 1.0

    row_of = _row_of_node(np.arange(N, dtype=np.int64), NP, WIN, C, CW)

    pr = Prep()
    pr.N, pr.C, pr.NP, pr.WIN = N, C, NP, WIN
    pr.AGC, pr.CW, pr.CBLK, pr.S = AGC, CW, CBLK, S

    # per-core, per-stream (== per-chunk) sorted message lists
    core_ld = [[None] * S for _ in range(C)]  # local dst per stream
    core_idx = [[None] * S for _ in range(C)]  # within-chunk table idx
    for i in range(C):
        m = (dst >= i * NP) & (dst < (i + 1) * NP)
        s_i = src[m]
        ld_i = dst[m] - i * NP
        order = np.argsort(ld_i, kind="stable")
        s_i, ld_i = s_i[order], ld_i[order]
        rows = row_of[s_i]
        cs = rows // CBLK
        for s in range(S):
            a = cs == s
            core_ld[i][s] = ld_i[a]
            core_idx[i][s] = rows[a] % CBLK

    # re-align all cores' streams at every RESYNC_G windows: within a group,
    # pad each core's segment to the max core's tile count. (G=WIN: only the
    # global per-stream tile count is equalized across cores.)
    G = CFG.get("RESYNC_G", 64)
    n_groups = (WIN + G - 1) // G
    for s in range(S):
        seg_tiles = np.zeros(n_groups, dtype=np.int64)
        for g in range(n_groups):
            lo, hi = g * G * 128, min((g + 1) * G, WIN) * 128
            for i in range(C):
                cnt = int(((core_ld[i][s] >= lo) & (core_ld[i][s] < hi)).sum())
                seg_tiles[g] = max(seg_tiles[g], (cnt + 127) // 128)
        for i in range(C):
            lds, ixs = [], []
            for g in range(n_groups):
                lo, hi = g * G * 128, min((g + 1) * G, WIN) * 128
                m = (core_ld[i][s] >= lo) & (core_ld[i][s] < hi)
                ld_g, ix_g = core_ld[i][s][m], core_idx[i][s][m]
                pad = int(seg_tiles[g]) * 128 - len(ld_g)
                lds.append(np.concatenate([ld_g, np.full(pad, SENT, np.int64)]))
                ixs.append(np.concatenate([ix_g, np.zeros(pad, np.int64)]))
            core_ld[i][s] = np.concatenate(lds)
            core_idx[i][s] = np.concatenate(ixs)
    T = [len(core_ld[0][s]) // 128 for s in range(S)]
    pr.T = T

    for i in range(C):
        for s in range(S):
            assert len(core_ld[i][s]) == T[s] * 128

    # union pair structure (w, stream, tile) across cores
    pair_set = set()
    for i in range(C):
        for s in range(S):
            L = core_ld[i][s].reshape(T[s], 128)
            for t in range(T[s]):
                real = L[t][L[t] != SENT]
                if len(real) == 0:
                    continue
                for w in range(int(real.min()) // 128, int(real.max()) // 128 + 1):
                    pair_set.add((w, s, t))
    for w in range(WIN):  # every window needs >=1 pair so psum gets reset
        if not any(p[0] == w for p in pair_set):
            pair_set.add((w, 0, 0))
    pairs = sorted(pair_set)
    pr.pairs = pairs
    pr.n_pairs = len(pairs)
    segs = [[] for _ in range(WIN)]
    for k, (w, s, t) in enumerate(pairs):
        segs[w].append(k)
    pr.segs = segs

    # per-core one-hot tiles [128, n_pairs, 128] fp8e4m3(0/1); the PE takes
    # fp8 stationary against bf16 moving directly, so no cast is needed
    pr.onehot = []
    pr.idx_wrapped = []
    pr.deg_staged = []
    FP8_ONE = 0x38
    for i in range(C):
        oh = np.zeros((128, pr.n_pairs, 128), dtype=np.uint8)
        for k, (w, s, t) in enumerate(pairs):
            ld_t = core_ld[i][s][t * 128 : (t + 1) * 128]
            slot = ld_t - 128 * w
            valid = (slot >= 0) & (slot < 128)
            rr = np.nonzero(valid)[0]
            oh[rr, k, slot[rr]] = FP8_ONE
        pr.onehot.append(oh.view(ml_dtypes.float8_e4m3fn))

        blocks = []
        for s in range(S):
            ix = core_idx[i][s].astype(np.int16)
            assert (core_idx[i][s] < CBLK).all() and (core_idx[i][s] >= 0).all()
            w16 = ix.reshape(-1, 16).T  # [16, T*8]
            blocks.append(np.tile(w16, (8, 1)))  # replicate to 128 partitions
        pr.idx_wrapped.append(
            np.ascontiguousarray(np.concatenate(blocks, axis=1))
        )

        dshard = deg[i * NP : (i + 1) * NP]
        pr.deg_staged.append(
            np.ascontiguousarray(dshard.reshape(WIN, 128).T.astype(np.float32))
        )

    return pr


# ------------------------------------------------------------------ bass ----


def _build(pr):
    import concourse.bass as bass
    import concourse.bacc as bacc
    import concourse.mybir as mybir
    import concourse.tile as tile
    from concourse._compat import get_trn_type

    dt = mybir.dt
    Alu = mybir.AluOpType
    F32, BF16, FP8, I16 = dt.float32, dt.bfloat16, dt.float8e4, dt.int16

    IN_C, OUT_C = CFG["IN_C"], CFG["OUT_C"]
    OC_PAD, CH, CHP = CFG["OC_PAD"], CFG["CH"], CFG["CHP"]
    N, C, NP, WIN = pr.N, pr.C, pr.NP, pr.WIN
    AGC, CW, CBLK, S = pr.AGC, pr.CW, pr.CBLK, pr.S
    CROWS = 128 * CW  # local rows per AG chunk
    T = pr.T
    NQ = min(S, 4)

    nc = bacc.Bacc(
        get_trn_type() or "TRN2",
        target_bir_lowering=False,
        debug=False,
        num_devices=C,
        num_swdge_queues=NQ,
    )

    xt_d = nc.dram_tensor("xt", [IN_C, NP], BF16, kind="ExternalInput")
    wt_d = nc.dram_tensor("wt", [IN_C, OUT_C], BF16, kind="ExternalInput")
    b_d = nc.dram_tensor("bias", [128, OUT_C], F32, kind="ExternalInput")
    deg_d = nc.dram_tensor("deg", [128, WIN], F32, kind="ExternalInput")
    idx_d = nc.dram_tensor(
        "idx", [128, sum(T) * 8], I16, kind="ExternalInput"
    )
    oh_d = nc.dram_tensor("oh", [128, pr.n_pairs, 128], FP8, kind="ExternalInput")
    out_d = nc.dram_tensor("out", [128, WIN, OUT_C], F32, kind="ExternalOutput")

    rg = [list(range(C))]

    with tile.TileContext(nc) as tc:
        with (
            tc.tile_pool(name="const", bufs=1) as const,
            tc.tile_pool(name="dram", bufs=1, space="DRAM") as dram,
            tc.tile_pool(name="psum_y", bufs=2, space="PSUM") as psum_y,
            tc.tile_pool(name="psum_w", bufs=CFG["PSUM_BUFS"], space="PSUM") as psum_w,
            tc.tile_pool(name="msg0", bufs=CFG["MSG_BUFS"]) as msg0_pool,
            tc.tile_pool(name="msg1", bufs=CFG["MSG_BUFS"]) as msg1_pool,
            tc.tile_pool(name="msg2", bufs=CFG["MSG_BUFS"]) as msg2_pool,
            tc.tile_pool(name="msg3", bufs=CFG["MSG_BUFS"]) as msg3_pool,
            tc.tile_pool(name="ohp", bufs=CFG["OH_BUFS"]) as oh_pool,
            tc.tile_pool(name="tmp", bufs=4) as tmp_pool,
        ):
            cc1_in = dram.tile([NP, OC_PAD], BF16)
            cc2_in = dram.tile([NP, OC_PAD], BF16)
            cc1_outs = [
                dram.tile(
                    [CBLK, OC_PAD], BF16, addr_space="Shared", name=f"cc1o{c}"
                )
                for c in range(AGC)
            ]
            cc2_outs = [
                dram.tile(
                    [CBLK, OC_PAD], BF16, addr_space="Shared", name=f"cc2o{c}"
                )
                for c in range(AGC)
            ]

            idx_sb = const.tile([128, sum(T) * 8], I16)
            nc.sync.dma_start(idx_sb[:], idx_d[:])
            wt_sb = const.tile([IN_C, OUT_C], BF16)
            nc.sync.dma_start(wt_sb[:], wt_d[:])
            b_sb = const.tile([128, OUT_C], F32)
            nc.sync.dma_start(b_sb[:], b_d[:])
            deg_sb = const.tile([128, WIN], F32)
            nc.sync.dma_start(deg_sb[:], deg_d[:])
            xt_sb = const.tile([IN_C, NP], BF16)
            nc.sync.dma_start(xt_sb[:], xt_d[:])

            deginv = const.tile([128, WIN], F32)
            nc.vector.reciprocal(deginv[:], deg_sb[:])
            dinv = const.tile([128, WIN], F32)
            nc.scalar.activation(
                dinv[:], deginv[:], mybir.ActivationFunctionType.Sqrt
            )

            z0f = const.tile([128, WIN, OUT_C], F32)
            z1f = const.tile([128, WIN, OUT_C], F32)
            outst = const.tile([128, WIN, OUT_C], F32)
            zpad1 = const.tile([128, WIN, OC_PAD], BF16)
            zpad2 = const.tile([128, WIN, OC_PAD], BF16)
            nc.vector.memset(zpad1[:], 0.0)
            nc.vector.memset(zpad2[:], 0.0)

            STAGE = CFG["STAGE"]

            stage_eng = nc.scalar if CFG.get("SCQ") else nc.sync

            def emit_ag(cc_in, cc_outs, c, zpad):
                stage_eng.dma_start(
                    cc_in[c * CROWS : (c + 1) * CROWS, :],
                    zpad[:, c * CW : (c + 1) * CW, :],
                )
                nc.gpsimd.collective_compute(
                    "AllGather",
                    Alu.bypass,
                    replica_groups=rg,
                    ins=[cc_in[c * CROWS : (c + 1) * CROWS, :].opt()],
                    outs=[cc_outs[c][:].opt()],
                )

            # ---- projection: z0 = dinv * (x @ W^T), staged [p, w, ch] ----
            for r in range(WIN):
                py = psum_y.tile([128, OUT_C], F32)
                nc.tensor.matmul(
                    py[:],
                    xt_sb[:, r * 128 : (r + 1) * 128],
                    wt_sb[:],
                    start=True,
                    stop=True,
                )
                if CFG["ACT_EVAC"]:
                    nc.scalar.mul(z0f[:, r, :], py[:], dinv[:, r : r + 1])
                    nc.scalar.copy(zpad1[:, r, 0:OUT_C], z0f[:, r, :])
                else:
                    nc.vector.tensor_scalar(
                        z0f[:, r, :], py[:], dinv[:, r : r + 1], None, Alu.mult
                    )
                    nc.vector.tensor_copy(zpad1[:, r, 0:OUT_C], z0f[:, r, :])
                if STAGE >= 2 and (r + 1) % CW == 0:
                    emit_ag(cc1_in, cc1_outs, r // CW, zpad1)

            calls = [(T[s] + CH - 1) // CH for s in range(S)]
            n_oh_chunks = (pr.n_pairs + CHP - 1) // CHP
            colbase = np.concatenate([[0], np.cumsum(np.array(T) * 8)])
            msg_pools = [msg0_pool, msg1_pool, msg2_pool, msg3_pool][:S]
            gq_counter = [0]

            def run_hop(cc_outs, evac, do_mm=True, after_window=None):
                tabs = [cc_outs[s][:] for s in range(S)]
                msg_tiles = [{} for _ in range(S)]
                oh_tiles = {}
                next_call = [0] * S
                next_oh = [0]

                def emit_gather(s):
                    c = next_call[s]
                    ntiles = min(CH, T[s] - c * CH)
                    ni = ntiles * 128
                    t = msg_pools[s].tile([128, CH, OC_PAD], BF16, tag=f"msg{s}")
                    sl = slice(
                        int(colbase[s]) + c * CH * 8,
                        int(colbase[s]) + c * CH * 8 + ntiles * 8,
                    )
                    nc.gpsimd.dma_gather(
                        t[:, 0:ntiles, :],
                        tabs[s],
                        idx_sb[:, sl],
                        ni,
                        ni,
                        OC_PAD,
                        single_packet=(ni <= 1024),
                        queue_num=(gq_counter[0] if CFG.get("QRR") else s) % NQ,
                    )
                    gq_counter[0] += 1
                    msg_tiles[s][c] = t
                    next_call[s] = c + 1

                def emit_oh():
                    k = next_oh[0]
                    npair = min(CHP, pr.n_pairs - k * CHP)
                    t = oh_pool.tile([128, CHP, 128], FP8, tag="oh")
                    nc.sync.dma_start(
                        out=t[:, 0:npair, :],
                        in_=oh_d[:, k * CHP : k * CHP + npair, :],
                    )
                    oh_tiles[k] = t
                    next_oh[0] = k + 1

                for w in range(WIN):
                    seg = pr.segs[w]
                    # make sure resources (plus prefetch) exist
                    for pk in seg:
                        _, s, t = pr.pairs[pk]
                        while next_call[s] <= min(
                            t // CH + CFG.get("PF", 1), calls[s] - 1
                        ):
                            emit_gather(s)
                        while next_oh[0] <= min(pk // CHP + 1, n_oh_chunks - 1):
                            emit_oh()
                    if not do_mm:
                        continue
                    pw = psum_w.tile([128, OUT_C], F32)
                    for j, pk in enumerate(seg):
                        _, s, t = pr.pairs[pk]
                        oh_ap = oh_tiles[pk // CHP][:, pk % CHP, :]
                        msg_ap = msg_tiles[s][t // CH][:, t % CH, 0:OUT_C]
                        nc.tensor.matmul(
                            pw[:],
                            oh_ap,
                            msg_ap,
                            start=(j == 0),
                            stop=(j == len(seg) - 1),
                        )
                    evac(w, pw)
                    if after_window is not None:
                        after_window(w)

            # ---- hop 1:  z1 = (psum + z0) / deg ----
            def evac1(w, pw):
                tmp = tmp_pool.tile([128, OUT_C], F32, tag="tmp")
                nc.vector.tensor_add(tmp[:], pw[:], z0f[:, w, :])
                if CFG["ACT_EVAC"]:
                    nc.scalar.mul(z1f[:, w, :], tmp[:], deginv[:, w : w + 1])
                    nc.scalar.mul(
                        zpad2[:, w, 0:OUT_C], tmp[:], deginv[:, w : w + 1]
                    )
                else:
                    nc.vector.tensor_scalar(
                        z1f[:, w, :], tmp[:], deginv[:, w : w + 1], None, Alu.mult
                    )
                    nc.vector.tensor_copy(zpad2[:, w, 0:OUT_C], z1f[:, w, :])

            agd = CFG.get("AGD", 0)
            ag2_fire = {
                min((c + 1) * CW - 1 + (agd if c < AGC - 1 else 0), WIN - 1): c
                for c in range(AGC)
            }

            def after1(w):
                if STAGE >= 5 and w in ag2_fire:
                    emit_ag(cc2_in, cc2_outs, ag2_fire[w], zpad2)

            if STAGE >= 3:
                run_hop(cc1_outs, evac1, do_mm=STAGE >= 4, after_window=after1)

            # ---- hop 2:  out = dinv * (psum + z1) + b ----
            def evac2(w, pw):
                tmp = tmp_pool.tile([128, OUT_C], F32, tag="tmp")
                tmp2 = tmp_pool.tile([128, OUT_C], F32, tag="tmp2")
                nc.vector.tensor_add(tmp[:], pw[:], z1f[:, w, :])
                if CFG["ACT_EVAC"]:
                    nc.scalar.mul(tmp2[:], tmp[:], dinv[:, w : w + 1])
                else:
                    nc.vector.tensor_scalar(
                        tmp2[:], tmp[:], dinv[:, w : w + 1], None, Alu.mult
                    )
                nc.vector.tensor_add(outst[:, w, :], tmp2[:], b_sb[:])

            def after2(w):
                if (w + 1) % CW == 0:
                    c = w // CW
                    stage_eng.dma_start(
                        out_d[:, c * CW : (c + 1) * CW, :],
                        outst[:, c * CW : (c + 1) * CW, :],
                    )

            if STAGE >= 6:
                run_hop(cc2_outs, evac2, after_window=after2)
            else:
                src_final = {1: z0f, 2: z0f, 3: z0f, 4: z1f, 5: z1f}[STAGE]
                nc.sync.dma_start(out_d[:], src_final[:])

    nc.compile()
    return nc


def _make_in_maps(pr, x, W, b):
    C, NP, WIN = pr.C, pr.NP, pr.WIN
    x = np.asarray(x, dtype=np.float32)
    W = np.asarray(W, dtype=np.float32)
    b = np.asarray(b, dtype=np.float32)
    wt = np.ascontiguousarray(W.T.astype(ml_dtypes.bfloat16))
    b_rep = np.ascontiguousarray(np.broadcast_to(b, (128, len(b))))
    in_maps = []
    for i in range(C):
        xt = np.ascontiguousarray(
            x[i * NP : (i + 1) * NP].T.astype(ml_dtypes.bfloat16)
        )
        in_maps.append(
            dict(
                xt=xt,
                wt=wt,
                bias=b_rep,
                deg=pr.deg_staged[i],
                idx=pr.idx_wrapped[i],
                oh=pr.onehot[i],
            )
        )
    return in_maps


def _unpermute(o, pr):
    # device layout is [p, w, ch]; node order is w*128+p
    return (
        o.reshape(128, pr.WIN, o.shape[-1])
        .transpose(1, 0, 2)
        .reshape(pr.NP, o.shape[-1])
    )


def kernel(x, edge_index, W, b):
    pr = _preprocess(edge_index)
    nc = _build(pr)
    in_maps = _make_in_maps(pr, x, W, b)

    from concourse import bass_utils

    res = bass_utils.run_bass_kernel_spmd(
        nc, in_maps, core_ids=list(range(pr.C))
    )
    shards = [_unpermute(res.results[i]["out"], pr) for i in range(pr.C)]
    return np.ascontiguousarray(np.concatenate(shards, axis=0))


# revision 49
# speedup vs baseline: 1.3922x; 1.1058x over previous
"""SGC (2-hop simple graph convolution) Trainium2 kernel, 8-core SPMD.

out = S S x W^T + b,  S = D^{-1/2} (A + I) D^{-1/2}   (D = in-degree + 1)

Strategy:
  * project first: y = x @ W^T (64 ch), exact by associativity
  * factor norms:  S z = dinv * (A+I) (dinv * z)  -> per-node scalings only,
    messages are unweighted; self loop handled as a local add
  * per core: own 1/8 of destination nodes; edges partitioned by dst
  * z tables are AllGather'ed in AGC chunks (chunk-major node permutation so
    every chunk is a contiguous Shared tensor); each chunk is its own gather
    "stream" pinned to its own SWDGE queue, so chunk-s gathers start as soon
    as chunk-s AG lands and descriptor generation runs 4-wide on gpsimd
  * gather sources with gpsimd dma_gather from the bf16 chunk tables
    (rows padded to 128 ch = 256 B to satisfy the elem%256 constraint);
    ni<=1024 per call enables single_packet
  * scatter-adds via PE matmul: 128-message tiles x host-built 0/1 one-hot
    stationary tiles (fp8, streamed on the sync HWDGE queue; the PE takes fp8
    stationary against bf16 moving directly), accumulated in PSUM per
    128-destination window; out-of-window slots give all-zero rows so stream
    tiles may straddle windows with no padding
  * node numbering inside tables is chunk-major permuted; the output staging
    keeps the [p, w] permutation and the host un-permutes at the end
"""

import sys

sys.path.insert(0, "/opt/trn_rl_repo")

import numpy as np
import ml_dtypes

# ---------------- problem constants (overridden by tests for small runs) ----
CFG = dict(
    N_NODES=65536,
    N_EDGES=655360,
    IN_C=128,
    OUT_C=64,
    CORES=8,
    CH=8,  # gather tiles (128 msgs each) per dma_gather call; ni<=1024 -> single_packet
    CHP=32,  # one-hot pairs per DMA chunk
    OC_PAD=128,  # bf16 channels per gather-table row (256 B)
    MSG_BUFS=8,  # per-stream message buffers
    OH_BUFS=3,
    PF=4,  # gather chunks prefetched ahead of demand
    PSUM_BUFS=6,
    RESYNC_G=64,  # no intra-stream alignment: fewest gather tiles wins
    ACT_EVAC=1,
    STAGE=6,  # debug: 1 proj, 2 +ag1, 3 +gather/oh, 4 +hop1 mm, 5 +ag2, 6 full
    AGC=4,  # AllGather chunks == gather streams == SWDGE queues
    QRR=1,  # 1: round-robin gather queue assignment; 0: queue = stream id
    AGD=2,  # windows to delay non-final AG2 triggers (lets staging finish)
    SCQ=1,  # 1: AG staging + out chunks on scalar HWDGE queue
)

SENT = 1 << 20  # sentinel "dst" for pad rows -> all-zero one-hot everywhere


class Prep:
    pass


def _row_of_node(n, NP, WIN, C, CW):
    # node n -> chunk-major gather-table row so each AG chunk is contiguous:
    # row = c*(C*128*CW) + core*(128*CW) + p*CW + w%CW   (p = r%128, w = r//128)
    r = n % NP
    p = r % 128
    w = r // 128
    c = w // CW
    wr = w % CW
    return c * (C * 128 * CW) + (n // NP) * (128 * CW) + p * CW + wr


def _preprocess(edge_index):
    N = CFG["N_NODES"]
    C = CFG["CORES"]
    NP = N // C
    WIN = NP // 128
    AGC = CFG["AGC"]
    CW = WIN // AGC
    CBLK = C * 128 * CW  # rows per chunk tensor
    S = AGC
    assert WIN % AGC == 0
    assert CBLK <= 32768  # int16 gather indices

    src = np.asarray(edge_index[0], dtype=np.int64)
    dst = np.asarray(edge_index[1], dtype=np.int64)
    deg = np.bincount(dst, minlength=N).astype(np.float32) + 1.0

    row_of = _row_of_node(np.arange(N, dtype=np.int64), NP, WIN, C, CW)

    pr = Prep()
    pr.N, pr.C, pr.NP, pr.WIN = N, C, NP, WIN
    pr.AGC, pr.CW, pr.CBLK, pr.S = AGC, CW, CBLK, S

    # per-core, per-stream (== per-chunk) sorted message lists
    core_ld = [[None] * S for _ in range(C)]  # local dst per stream
    core_idx = [[None] * S for _ in range(C)]  # within-chunk table idx
    for i in range(C):
        m = (dst >= i * NP) & (dst < (i + 1) * NP)
        s_i = src[m]
        ld_i = dst[m] - i * NP
        order = np.argsort(ld_i, kind="stable")
        s_i, ld_i = s_i[order], ld_i[order]
        rows = row_of[s_i]
        cs = rows // CBLK
        for s in range(S):
            a = cs == s
            core_ld[i][s] = ld_i[a]
            core_idx[i][s] = rows[a] % CBLK

    # re-align all cores' streams at every RESYNC_G windows: within a group,
    # pad each core's segment to the max core's tile count. (G=WIN: only the
    # global per-stream tile count is equalized across cores.)
    G = CFG.get("RESYNC_G", 64)
    n_groups = (WIN + G - 1) // G
    for s in range(S):
        seg_tiles = np.zeros(n_groups, dtype=np.int64)
        for g in range(n_groups):
            lo, hi = g * G * 128, min((g + 1) * G, WIN) * 128
            for i in range(C):
                cnt = int(((core_ld[i][s] >= lo) & (core_ld[i][s] < hi)).sum())
                seg_tiles[g] = max(seg_tiles[g], (cnt + 127) // 128)
        for i in range(C):
            lds, ixs = [], []
            for g in range(n_groups):
                lo, hi = g * G * 128, min((g + 1) * G, WIN) * 128
                m = (core_ld[i][s] >= lo) & (core_ld[i][s] < hi)
                ld_g, ix_g = core_ld[i][s][m], core_idx[i][s][m]
                pad = int(seg_tiles[g]) * 128 - len(ld_g)
                lds.append(np.concatenate([ld_g, np.full(pad, SENT, np.int64)]))
                ixs.append(np.concatenate([ix_g, np.zeros(pad, np.int64)]))
            core_ld[i][s] = np.concatenate(lds)
            core_idx[i][s] = np.concatenate(ixs)
    T = [len(core_ld[0][s]) // 128 for s in range(S)]
    pr.T = T

    for i in range(C):
        for s in range(S):
            assert len(core_ld[i][s]) == T[s] * 128

    # union pair structure (w, stream, tile) across cores
    pair_set = set()
    for i in range(C):
        for s in range(S):
            L = core_ld[i][s].reshape(T[s], 128)
            for t in range(T[s]):
                real = L[t][L[t] != SENT]
                if len(real) == 0:
                    continue
                for w in range(int(real.min()) // 128, int(real.max()) // 128 + 1):
                    pair_set.add((w, s, t))
    for w in range(WIN):  # every window needs >=1 pair so psum gets reset
        if not any(p[0] == w for p in pair_set):
            pair_set.add((w, 0, 0))
    pairs = sorted(pair_set)
    pr.pairs = pairs
    pr.n_pairs = len(pairs)
    segs = [[] for _ in range(WIN)]
    for k, (w, s, t) in enumerate(pairs):
        segs[w].append(k)
    pr.segs = segs

    # per-core one-hot tiles [128, n_pairs, 128] fp8e4m3(0/1); the PE takes
    # fp8 stationary against bf16 moving directly, so no cast is needed
    pr.onehot = []
    pr.idx_wrapped = []
    pr.deg_staged = []
    FP8_ONE = 0x38
    for i in range(C):
        oh = np.zeros((128, pr.n_pairs, 128), dtype=np.uint8)
        for k, (w, s, t) in enumerate(pairs):
            ld_t = core_ld[i][s][t * 128 : (t + 1) * 128]
            slot = ld_t - 128 * w
            valid = (slot >= 0) & (slot < 128)
            rr = np.nonzero(valid)[0]
            oh[rr, k, slot[rr]] = FP8_ONE
        pr.onehot.append(oh.view(ml_dtypes.float8_e4m3fn))

        blocks = []
        for s in range(S):
            ix = core_idx[i][s].astype(np.int16)
            assert (core_idx[i][s] < CBLK).all() and (core_idx[i][s] >= 0).all()
            w16 = ix.reshape(-1, 16).T  # [16, T*8]
            blocks.append(np.tile(w16, (8, 1)))  # replicate to 128 partitions
        pr.idx_wrapped.append(
            np.ascontiguousarray(np.concatenate(blocks, axis=1))
        )

        dshard = deg[i * NP : (i + 1) * NP]
        pr.deg_staged.append(
            np.ascontiguousarray(dshard.reshape(WIN, 128).T.astype(np.float32))
        )

    return pr


# ------------------------------------------------------------------ bass ----


def _build(pr):
    import concourse.bass as bass
    import concourse.bacc as bacc
    import concourse.mybir as mybir
    import concourse.tile as tile
    from concourse._compat import get_trn_type

    dt = mybir.dt
    Alu = mybir.AluOpType
    F32, BF16, FP8, I16 = dt.float32, dt.bfloat16, dt.float8e4, dt.int16

    IN_C, OUT_C = CFG["IN_C"], CFG["OUT_C"]
    OC_PAD, CH, CHP = CFG["OC_PAD"], CFG["CH"], CFG["CHP"]
    N, C, NP, WIN = pr.N, pr.C, pr.NP, pr.WIN
    AGC, CW, CBLK, S = pr.AGC, pr.CW, pr.CBLK, pr.S
    CROWS = 128 * CW  # local rows per AG chunk
    T = pr.T
    NQ = 4 if CFG.get("QRR") else min(S, 4)

    nc = bacc.Bacc(
        get_trn_type() or "TRN2",
        target_bir_lowering=False,
        debug=False,
        num_devices=C,
        num_swdge_queues=NQ,
    )

    xt_d = nc.dram_tensor("xt", [IN_C, NP], BF16, kind="ExternalInput")
    wt_d = nc.dram_tensor("wt", [IN_C, OUT_C], BF16, kind="ExternalInput")
    b_d = nc.dram_tensor("bias", [128, OUT_C], F32, kind="ExternalInput")
    deg_d = nc.dram_tensor("deg", [128, WIN], F32, kind="ExternalInput")
    idx_d = nc.dram_tensor(
        "idx", [128, sum(T) * 8], I16, kind="ExternalInput"
    )
    oh_d = nc.dram_tensor("oh", [128, pr.n_pairs, 128], FP8, kind="ExternalInput")
    out_d = nc.dram_tensor("out", [128, WIN, OUT_C], F32, kind="ExternalOutput")

    rg = [list(range(C))]

    with tile.TileContext(nc) as tc:
        with (
            tc.tile_pool(name="const", bufs=1) as const,
            tc.tile_pool(name="dram", bufs=1, space="DRAM") as dram,
            tc.tile_pool(name="psum_y", bufs=2, space="PSUM") as psum_y,
            tc.tile_pool(name="psum_w", bufs=CFG["PSUM_BUFS"], space="PSUM") as psum_w,
            tc.tile_pool(name="msg0", bufs=CFG["MSG_BUFS"]) as msg0_pool,
            tc.tile_pool(name="msg1", bufs=CFG["MSG_BUFS"]) as msg1_pool,
            tc.tile_pool(name="msg2", bufs=CFG["MSG_BUFS"]) as msg2_pool,
            tc.tile_pool(name="msg3", bufs=CFG["MSG_BUFS"]) as msg3_pool,
            tc.tile_pool(name="ohp", bufs=CFG["OH_BUFS"]) as oh_pool,
            tc.tile_pool(name="tmp", bufs=4) as tmp_pool,
        ):
            cc1_in = dram.tile([NP, OC_PAD], BF16)
            cc2_in = dram.tile([NP, OC_PAD], BF16)
            cc1_outs = [
                dram.tile(
                    [CBLK, OC_PAD], BF16, addr_space="Shared", name=f"cc1o{c}"
                )
                for c in range(AGC)
            ]
            cc2_outs = [
                dram.tile(
                    [CBLK, OC_PAD], BF16, addr_space="Shared", name=f"cc2o{c}"
                )
                for c in range(AGC)
            ]

            idx_sb = const.tile([128, sum(T) * 8], I16)
            nc.sync.dma_start(idx_sb[:], idx_d[:])
            wt_sb = const.tile([IN_C, OUT_C], BF16)
            nc.sync.dma_start(wt_sb[:], wt_d[:])
            b_sb = const.tile([128, OUT_C], F32)
            nc.sync.dma_start(b_sb[:], b_d[:])
            deg_sb = const.tile([128, WIN], F32)
            nc.sync.dma_start(deg_sb[:], deg_d[:])
            xt_sb = const.tile([IN_C, NP], BF16)
            nc.sync.dma_start(xt_sb[:], xt_d[:])

            deginv = const.tile([128, WIN], F32)
            nc.vector.reciprocal(deginv[:], deg_sb[:])
            dinv = const.tile([128, WIN], F32)
            nc.scalar.activation(
                dinv[:], deginv[:], mybir.ActivationFunctionType.Sqrt
            )

            z0f = const.tile([128, WIN, OUT_C], F32)
            z1f = const.tile([128, WIN, OUT_C], F32)
            outst = const.tile([128, WIN, OUT_C], F32)
            zpad1 = const.tile([128, WIN, OC_PAD], BF16)
            zpad2 = const.tile([128, WIN, OC_PAD], BF16)
            nc.vector.memset(zpad1[:], 0.0)
            nc.vector.memset(zpad2[:], 0.0)

            STAGE = CFG["STAGE"]

            stage_eng = nc.scalar if CFG.get("SCQ") else nc.sync

            def emit_ag(cc_in, cc_outs, c, zpad):
                stage_eng.dma_start(
                    cc_in[c * CROWS : (c + 1) * CROWS, :],
                    zpad[:, c * CW : (c + 1) * CW, :],
                )
                nc.gpsimd.collective_compute(
                    "AllGather",
                    Alu.bypass,
                    replica_groups=rg,
                    ins=[cc_in[c * CROWS : (c + 1) * CROWS, :].opt()],
                    outs=[cc_outs[c][:].opt()],
                )

            # ---- projection: z0 = dinv * (x @ W^T), staged [p, w, ch] ----
            for r in range(WIN):
                py = psum_y.tile([128, OUT_C], F32)
                nc.tensor.matmul(
                    py[:],
                    xt_sb[:, r * 128 : (r + 1) * 128],
                    wt_sb[:],
                    start=True,
                    stop=True,
                )
                if CFG["ACT_EVAC"]:
                    nc.scalar.mul(z0f[:, r, :], py[:], dinv[:, r : r + 1])
                    nc.scalar.copy(zpad1[:, r, 0:OUT_C], z0f[:, r, :])
                else:
                    nc.vector.tensor_scalar(
                        z0f[:, r, :], py[:], dinv[:, r : r + 1], None, Alu.mult
                    )
                    nc.vector.tensor_copy(zpad1[:, r, 0:OUT_C], z0f[:, r, :])
                if STAGE >= 2 and (r + 1) % CW == 0:
                    emit_ag(cc1_in, cc1_outs, r // CW, zpad1)

            calls = [(T[s] + CH - 1) // CH for s in range(S)]
            n_oh_chunks = (pr.n_pairs + CHP - 1) // CHP
            colbase = np.concatenate([[0], np.cumsum(np.array(T) * 8)])
            msg_pools = [msg0_pool, msg1_pool, msg2_pool, msg3_pool][:S]
            gq_counter = [0]

            def run_hop(cc_outs, evac, do_mm=True, after_window=None):
                tabs = [cc_outs[s][:] for s in range(S)]
                msg_tiles = [{} for _ in range(S)]
                oh_tiles = {}
                next_call = [0] * S
                next_oh = [0]

                def emit_gather(s):
                    c = next_call[s]
                    ntiles = min(CH, T[s] - c * CH)
                    ni = ntiles * 128
                    t = msg_pools[s].tile([128, CH, OC_PAD], BF16, tag=f"msg{s}")
                    sl = slice(
                        int(colbase[s]) + c * CH * 8,
                        int(colbase[s]) + c * CH * 8 + ntiles * 8,
                    )
                    nc.gpsimd.dma_gather(
                        t[:, 0:ntiles, :],
                        tabs[s],
                        idx_sb[:, sl],
                        ni,
                        ni,
                        OC_PAD,
                        single_packet=(ni <= 1024),
                        queue_num=(gq_counter[0] if CFG.get("QRR") else s) % NQ,
                    )
                    gq_counter[0] += 1
                    msg_tiles[s][c] = t
                    next_call[s] = c + 1

                def emit_oh():
                    k = next_oh[0]
                    npair = min(CHP, pr.n_pairs - k * CHP)
                    t = oh_pool.tile([128, CHP, 128], FP8, tag="oh")
                    nc.sync.dma_start(
                        out=t[:, 0:npair, :],
                        in_=oh_d[:, k * CHP : k * CHP + npair, :],
                    )
                    oh_tiles[k] = t
                    next_oh[0] = k + 1

                for w in range(WIN):
                    seg = pr.segs[w]
                    # make sure resources (plus prefetch) exist
                    for pk in seg:
                        _, s, t = pr.pairs[pk]
                        while next_call[s] <= min(
                            t // CH + CFG.get("PF", 1), calls[s] - 1
                        ):
                            emit_gather(s)
                        while next_oh[0] <= min(pk // CHP + 1, n_oh_chunks - 1):
                            emit_oh()
                    if not do_mm:
                        continue
                    pw = psum_w.tile([128, OUT_C], F32)
                    for j, pk in enumerate(seg):
                        _, s, t = pr.pairs[pk]
                        oh_ap = oh_tiles[pk // CHP][:, pk % CHP, :]
                        msg_ap = msg_tiles[s][t // CH][:, t % CH, 0:OUT_C]
                        nc.tensor.matmul(
                            pw[:],
                            oh_ap,
                            msg_ap,
                            start=(j == 0),
                            stop=(j == len(seg) - 1),
                        )
                    evac(w, pw)
                    if after_window is not None:
                        after_window(w)

            # ---- hop 1:  z1 = (psum + z0) / deg ----
            def evac1(w, pw):
                tmp = tmp_pool.tile([128, OUT_C], F32, tag="tmp")
                nc.vector.tensor_add(tmp[:], pw[:], z0f[:, w, :])
                if CFG["ACT_EVAC"]:
                    nc.scalar.mul(z1f[:, w, :], tmp[:], deginv[:, w : w + 1])
                    nc.scalar.mul(
                        zpad2[:, w, 0:OUT_C], tmp[:], deginv[:, w : w + 1]
                    )
                else:
                    nc.vector.tensor_scalar(
                        z1f[:, w, :], tmp[:], deginv[:, w : w + 1], None, Alu.mult
                    )
                    nc.vector.tensor_copy(zpad2[:, w, 0:OUT_C], z1f[:, w, :])

            agd = CFG.get("AGD", 0)
            ag2_fire = {
                min((c + 1) * CW - 1 + (agd if c < AGC - 1 else 0), WIN - 1): c
                for c in range(AGC)
            }

            def after1(w):
                if STAGE >= 5 and w in ag2_fire:
                    emit_ag(cc2_in, cc2_outs, ag2_fire[w], zpad2)

            if STAGE >= 3:
                run_hop(cc1_outs, evac1, do_mm=STAGE >= 4, after_window=after1)

            # ---- hop 2:  out = dinv * (psum + z1) + b ----
            def evac2(w, pw):
                tmp = tmp_pool.tile([128, OUT_C], F32, tag="tmp")
                tmp2 = tmp_pool.tile([128, OUT_C], F32, tag="tmp2")
                nc.vector.tensor_add(tmp[:], pw[:], z1f[:, w, :])
                if CFG["ACT_EVAC"]:
                    nc.scalar.mul(tmp2[:], tmp[:], dinv[:, w : w + 1])
                else:
                    nc.vector.tensor_scalar(
                        tmp2[:], tmp[:], dinv[:, w : w + 1], None, Alu.mult
                    )
                nc.vector.tensor_add(outst[:, w, :], tmp2[:], b_sb[:])

            def after2(w):
                if (w + 1) % CW == 0:
                    c = w // CW
                    stage_eng.dma_start(
                        out_d[:, c * CW : (c + 1) * CW, :],
                        outst[:, c * CW : (c + 1) * CW, :],
                    )

            if STAGE >= 6:
                run_hop(cc2_outs, evac2, after_window=after2)
            else:
                src_final = {1: z0f, 2: z0f, 3: z0f, 4: z1f, 5: z1f}[STAGE]
                nc.sync.dma_start(out_d[:], src_final[:])

    nc.compile()
    return nc


def _make_in_maps(pr, x, W, b):
    C, NP, WIN = pr.C, pr.NP, pr.WIN
    x = np.asarray(x, dtype=np.float32)
    W = np.asarray(W, dtype=np.float32)
    b = np.asarray(b, dtype=np.float32)
    wt = np.ascontiguousarray(W.T.astype(ml_dtypes.bfloat16))
    b_rep = np.ascontiguousarray(np.broadcast_to(b, (128, len(b))))
    in_maps = []
    for i in range(C):
        xt = np.ascontiguousarray(
            x[i * NP : (i + 1) * NP].T.astype(ml_dtypes.bfloat16)
        )
        in_maps.append(
            dict(
                xt=xt,
                wt=wt,
                bias=b_rep,
                deg=pr.deg_staged[i],
                idx=pr.idx_wrapped[i],
                oh=pr.onehot[i],
            )
        )
    return in_maps


def _unpermute(o, pr):
    # device layout is [p, w, ch]; node order is w*128+p
    return (
        o.reshape(128, pr.WIN, o.shape[-1])
        .transpose(1, 0, 2)
        .reshape(pr.NP, o.shape[-1])
    )


def kernel(x, edge_index, W, b):
    pr = _preprocess(edge_index)
    nc = _build(pr)
    in_maps = _make_in_maps(pr, x, W, b)

    from concourse import bass_utils

    res = bass_utils.run_bass_kernel_spmd(
        nc, in_maps, core_ids=list(range(pr.C))
    )
    shards = [_unpermute(res.results[i]["out"], pr) for i in range(pr.C)]
    return np.ascontiguousarray(np.concatenate(shards, axis=0))


# revision 50
# speedup vs baseline: 1.3951x; 1.0021x over previous
"""SGC (2-hop simple graph convolution) Trainium2 kernel, 8-core SPMD.

out = S S x W^T + b,  S = D^{-1/2} (A + I) D^{-1/2}   (D = in-degree + 1)

Strategy:
  * project first: y = x @ W^T (64 ch), exact by associativity
  * factor norms:  S z = dinv * (A+I) (dinv * z)  -> per-node scalings only,
    messages are unweighted; self loop handled as a local add
  * per core: own 1/8 of destination nodes; edges partitioned by dst
  * z tables are AllGather'ed in AGC chunks (chunk-major node permutation so
    every chunk is a contiguous Shared tensor); each chunk is its own gather
    "stream" pinned to its own SWDGE queue, so chunk-s gathers start as soon
    as chunk-s AG lands and descriptor generation runs 4-wide on gpsimd
  * gather sources with gpsimd dma_gather from the bf16 chunk tables
    (rows padded to 128 ch = 256 B to satisfy the elem%256 constraint);
    ni<=1024 per call enables single_packet
  * scatter-adds via PE matmul: 128-message tiles x host-built 0/1 one-hot
    stationary tiles (fp8, streamed on the sync HWDGE queue; the PE takes fp8
    stationary against bf16 moving directly), accumulated in PSUM per
    128-destination window; out-of-window slots give all-zero rows so stream
    tiles may straddle windows with no padding
  * node numbering inside tables is chunk-major permuted; the output staging
    keeps the [p, w] permutation and the host un-permutes at the end
"""

import sys

sys.path.insert(0, "/opt/trn_rl_repo")

import numpy as np
import ml_dtypes

# ---------------- problem constants (overridden by tests for small runs) ----
CFG = dict(
    N_NODES=65536,
    N_EDGES=655360,
    IN_C=128,
    OUT_C=64,
    CORES=8,
    CH=8,  # gather tiles (128 msgs each) per dma_gather call; ni<=1024 -> single_packet
    CHP=32,  # one-hot pairs per DMA chunk
    OC_PAD=128,  # bf16 channels per gather-table row (256 B)
    MSG_BUFS=16,  # per-stream message buffers
    OH_BUFS=3,
    PF=8,  # gather chunks prefetched ahead of demand
    PSUM_BUFS=6,
    RESYNC_G=64,  # no intra-stream alignment: fewest gather tiles wins
    ACT_EVAC=1,
    STAGE=6,  # debug: 1 proj, 2 +ag1, 3 +gather/oh, 4 +hop1 mm, 5 +ag2, 6 full
    AGC=2,  # AllGather chunks == gather streams (4 SWDGE queues via QRR)
    QRR=1,  # 1: round-robin gather queue assignment; 0: queue = stream id
    AGD=2,  # windows to delay non-final AG2 triggers (lets staging finish)
    SCQ=1,  # 1: AG staging + out chunks on scalar HWDGE queue
)

SENT = 1 << 20  # sentinel "dst" for pad rows -> all-zero one-hot everywhere


class Prep:
    pass


def _row_of_node(n, NP, WIN, C, CW):
    # node n -> chunk-major gather-table row so each AG chunk is contiguous:
    # row = c*(C*128*CW) + core*(128*CW) + p*CW + w%CW   (p = r%128, w = r//128)
    r = n % NP
    p = r % 128
    w = r // 128
    c = w // CW
    wr = w % CW
    return c * (C * 128 * CW) + (n // NP) * (128 * CW) + p * CW + wr


def _preprocess(edge_index):
    N = CFG["N_NODES"]
    C = CFG["CORES"]
    NP = N // C
    WIN = NP // 128
    AGC = CFG["AGC"]
    CW = WIN // AGC
    CBLK = C * 128 * CW  # rows per chunk tensor
    S = AGC
    assert WIN % AGC == 0
    assert CBLK <= 32768  # int16 gather indices

    src = np.asarray(edge_index[0], dtype=np.int64)
    dst = np.asarray(edge_index[1], dtype=np.int64)
    deg = np.bincount(dst, minlength=N).astype(np.float32) + 1.0

    row_of = _row_of_node(np.arange(N, dtype=np.int64), NP, WIN, C, CW)

    pr = Prep()
    pr.N, pr.C, pr.NP, pr.WIN = N, C, NP, WIN
    pr.AGC, pr.CW, pr.CBLK, pr.S = AGC, CW, CBLK, S

    # per-core, per-stream (== per-chunk) sorted message lists
    core_ld = [[None] * S for _ in range(C)]  # local dst per stream
    core_idx = [[None] * S for _ in range(C)]  # within-chunk table idx
    for i in range(C):
        m = (dst >= i * NP) & (dst < (i + 1) * NP)
        s_i = src[m]
        ld_i = dst[m] - i * NP
        order = np.argsort(ld_i, kind="stable")
        s_i, ld_i = s_i[order], ld_i[order]
        rows = row_of[s_i]
        cs = rows // CBLK
        for s in range(S):
            a = cs == s
            core_ld[i][s] = ld_i[a]
            core_idx[i][s] = rows[a] % CBLK

    # re-align all cores' streams at every RESYNC_G windows: within a group,
    # pad each core's segment to the max core's tile count. (G=WIN: only the
    # global per-stream tile count is equalized across cores.)
    G = CFG.get("RESYNC_G", 64)
    n_groups = (WIN + G - 1) // G
    for s in range(S):
        seg_tiles = np.zeros(n_groups, dtype=np.int64)
        for g in range(n_groups):
            lo, hi = g * G * 128, min((g + 1) * G, WIN) * 128
            for i in range(C):
                cnt = int(((core_ld[i][s] >= lo) & (core_ld[i][s] < hi)).sum())
                seg_tiles[g] = max(seg_tiles[g], (cnt + 127) // 128)
        for i in range(C):
            lds, ixs = [], []
            for g in range(n_groups):
                lo, hi = g * G * 128, min((g + 1) * G, WIN) * 128
                m = (core_ld[i][s] >= lo) & (core_ld[i][s] < hi)
                ld_g, ix_g = core_ld[i][s][m], core_idx[i][s][m]
                pad = int(seg_tiles[g]) * 128 - len(ld_g)
                lds.append(np.concatenate([ld_g, np.full(pad, SENT, np.int64)]))
                ixs.append(np.concatenate([ix_g, np.zeros(pad, np.int64)]))
            core_ld[i][s] = np.concatenate(lds)
            core_idx[i][s] = np.concatenate(ixs)
    T = [len(core_ld[0][s]) // 128 for s in range(S)]
    pr.T = T

    for i in range(C):
        for s in range(S):
            assert len(core_ld[i][s]) == T[s] * 128

    # union pair structure (w, stream, tile) across cores
    pair_set = set()
    for i in range(C):
        for s in range(S):
            L = core_ld[i][s].reshape(T[s], 128)
            for t in range(T[s]):
                real = L[t][L[t] != SENT]
                if len(real) == 0:
                    continue
                for w in range(int(real.min()) // 128, int(real.max()) // 128 + 1):
                    pair_set.add((w, s, t))
    for w in range(WIN):  # every window needs >=1 pair so psum gets reset
        if not any(p[0] == w for p in pair_set):
            pair_set.add((w, 0, 0))
    pairs = sorted(pair_set)
    pr.pairs = pairs
    pr.n_pairs = len(pairs)
    segs = [[] for _ in range(WIN)]
    for k, (w, s, t) in enumerate(pairs):
        segs[w].append(k)
    pr.segs = segs

    # per-core one-hot tiles [128, n_pairs, 128] fp8e4m3(0/1); the PE takes
    # fp8 stationary against bf16 moving directly, so no cast is needed
    pr.onehot = []
    pr.idx_wrapped = []
    pr.deg_staged = []
    FP8_ONE = 0x38
    for i in range(C):
        oh = np.zeros((128, pr.n_pairs, 128), dtype=np.uint8)
        for k, (w, s, t) in enumerate(pairs):
            ld_t = core_ld[i][s][t * 128 : (t + 1) * 128]
            slot = ld_t - 128 * w
            valid = (slot >= 0) & (slot < 128)
            rr = np.nonzero(valid)[0]
            oh[rr, k, slot[rr]] = FP8_ONE
        pr.onehot.append(oh.view(ml_dtypes.float8_e4m3fn))

        blocks = []
        for s in range(S):
            ix = core_idx[i][s].astype(np.int16)
            assert (core_idx[i][s] < CBLK).all() and (core_idx[i][s] >= 0).all()
            w16 = ix.reshape(-1, 16).T  # [16, T*8]
            blocks.append(np.tile(w16, (8, 1)))  # replicate to 128 partitions
        pr.idx_wrapped.append(
            np.ascontiguousarray(np.concatenate(blocks, axis=1))
        )

        dshard = deg[i * NP : (i + 1) * NP]
        pr.deg_staged.append(
            np.ascontiguousarray(dshard.reshape(WIN, 128).T.astype(np.float32))
        )

    return pr


# ------------------------------------------------------------------ bass ----


def _build(pr):
    import concourse.bass as bass
    import concourse.bacc as bacc
    import concourse.mybir as mybir
    import concourse.tile as tile
    from concourse._compat import get_trn_type

    dt = mybir.dt
    Alu = mybir.AluOpType
    F32, BF16, FP8, I16 = dt.float32, dt.bfloat16, dt.float8e4, dt.int16

    IN_C, OUT_C = CFG["IN_C"], CFG["OUT_C"]
    OC_PAD, CH, CHP = CFG["OC_PAD"], CFG["CH"], CFG["CHP"]
    N, C, NP, WIN = pr.N, pr.C, pr.NP, pr.WIN
    AGC, CW, CBLK, S = pr.AGC, pr.CW, pr.CBLK, pr.S
    CROWS = 128 * CW  # local rows per AG chunk
    T = pr.T
    NQ = 4 if CFG.get("QRR") else min(S, 4)

    nc = bacc.Bacc(
        get_trn_type() or "TRN2",
        target_bir_lowering=False,
        debug=False,
        num_devices=C,
        num_swdge_queues=NQ,
    )

    xt_d = nc.dram_tensor("xt", [IN_C, NP], BF16, kind="ExternalInput")
    wt_d = nc.dram_tensor("wt", [IN_C, OUT_C], BF16, kind="ExternalInput")
    b_d = nc.dram_tensor("bias", [128, OUT_C], F32, kind="ExternalInput")
    deg_d = nc.dram_tensor("deg", [128, WIN], F32, kind="ExternalInput")
    idx_d = nc.dram_tensor(
        "idx", [128, sum(T) * 8], I16, kind="ExternalInput"
    )
    oh_d = nc.dram_tensor("oh", [128, pr.n_pairs, 128], FP8, kind="ExternalInput")
    out_d = nc.dram_tensor("out", [128, WIN, OUT_C], F32, kind="ExternalOutput")

    rg = [list(range(C))]

    with tile.TileContext(nc) as tc:
        with (
            tc.tile_pool(name="const", bufs=1) as const,
            tc.tile_pool(name="dram", bufs=1, space="DRAM") as dram,
            tc.tile_pool(name="psum_y", bufs=2, space="PSUM") as psum_y,
            tc.tile_pool(name="psum_w", bufs=CFG["PSUM_BUFS"], space="PSUM") as psum_w,
            tc.tile_pool(name="msg0", bufs=CFG["MSG_BUFS"]) as msg0_pool,
            tc.tile_pool(name="msg1", bufs=CFG["MSG_BUFS"]) as msg1_pool,
            tc.tile_pool(name="msg2", bufs=CFG["MSG_BUFS"]) as msg2_pool,
            tc.tile_pool(name="msg3", bufs=CFG["MSG_BUFS"]) as msg3_pool,
            tc.tile_pool(name="ohp", bufs=CFG["OH_BUFS"]) as oh_pool,
            tc.tile_pool(name="tmp", bufs=4) as tmp_pool,
        ):
            cc1_in = dram.tile([NP, OC_PAD], BF16)
            cc2_in = dram.tile([NP, OC_PAD], BF16)
            cc1_outs = [
                dram.tile(
                    [CBLK, OC_PAD], BF16, addr_space="Shared", name=f"cc1o{c}"
                )
                for c in range(AGC)
            ]
            cc2_outs = [
                dram.tile(
                    [CBLK, OC_PAD], BF16, addr_space="Shared", name=f"cc2o{c}"
                )
                for c in range(AGC)
            ]

            idx_sb = const.tile([128, sum(T) * 8], I16)
            nc.sync.dma_start(idx_sb[:], idx_d[:])
            wt_sb = const.tile([IN_C, OUT_C], BF16)
            nc.sync.dma_start(wt_sb[:], wt_d[:])
            b_sb = const.tile([128, OUT_C], F32)
            nc.sync.dma_start(b_sb[:], b_d[:])
            deg_sb = const.tile([128, WIN], F32)
            nc.sync.dma_start(deg_sb[:], deg_d[:])
            xt_sb = const.tile([IN_C, NP], BF16)
            nc.sync.dma_start(xt_sb[:], xt_d[:])

            deginv = const.tile([128, WIN], F32)
            nc.vector.reciprocal(deginv[:], deg_sb[:])
            dinv = const.tile([128, WIN], F32)
            nc.scalar.activation(
                dinv[:], deginv[:], mybir.ActivationFunctionType.Sqrt
            )

            z0f = const.tile([128, WIN, OUT_C], F32)
            z1f = const.tile([128, WIN, OUT_C], F32)
            outst = const.tile([128, WIN, OUT_C], F32)
            zpad1 = const.tile([128, WIN, OC_PAD], BF16)
            zpad2 = const.tile([128, WIN, OC_PAD], BF16)
            nc.vector.memset(zpad1[:], 0.0)
            nc.vector.memset(zpad2[:], 0.0)

            STAGE = CFG["STAGE"]

            stage_eng = nc.scalar if CFG.get("SCQ") else nc.sync

            def emit_ag(cc_in, cc_outs, c, zpad):
                stage_eng.dma_start(
                    cc_in[c * CROWS : (c + 1) * CROWS, :],
                    zpad[:, c * CW : (c + 1) * CW, :],
                )
                nc.gpsimd.collective_compute(
                    "AllGather",
                    Alu.bypass,
                    replica_groups=rg,
                    ins=[cc_in[c * CROWS : (c + 1) * CROWS, :].opt()],
                    outs=[cc_outs[c][:].opt()],
                )

            # ---- projection: z0 = dinv * (x @ W^T), staged [p, w, ch] ----
            for r in range(WIN):
                py = psum_y.tile([128, OUT_C], F32)
                nc.tensor.matmul(
                    py[:],
                    xt_sb[:, r * 128 : (r + 1) * 128],
                    wt_sb[:],
                    start=True,
                    stop=True,
                )
                if CFG["ACT_EVAC"]:
                    nc.scalar.mul(z0f[:, r, :], py[:], dinv[:, r : r + 1])
                    nc.scalar.copy(zpad1[:, r, 0:OUT_C], z0f[:, r, :])
                else:
                    nc.vector.tensor_scalar(
                        z0f[:, r, :], py[:], dinv[:, r : r + 1], None, Alu.mult
                    )
                    nc.vector.tensor_copy(zpad1[:, r, 0:OUT_C], z0f[:, r, :])
                if STAGE >= 2 and (r + 1) % CW == 0:
                    emit_ag(cc1_in, cc1_outs, r // CW, zpad1)

            calls = [(T[s] + CH - 1) // CH for s in range(S)]
            n_oh_chunks = (pr.n_pairs + CHP - 1) // CHP
            colbase = np.concatenate([[0], np.cumsum(np.array(T) * 8)])
            msg_pools = [msg0_pool, msg1_pool, msg2_pool, msg3_pool][:S]
            gq_counter = [0]

            def run_hop(cc_outs, evac, do_mm=True, after_window=None):
                tabs = [cc_outs[s][:] for s in range(S)]
                msg_tiles = [{} for _ in range(S)]
                oh_tiles = {}
                next_call = [0] * S
                next_oh = [0]

                def emit_gather(s):
                    c = next_call[s]
                    ntiles = min(CH, T[s] - c * CH)
                    ni = ntiles * 128
                    t = msg_pools[s].tile([128, CH, OC_PAD], BF16, tag=f"msg{s}")
                    sl = slice(
                        int(colbase[s]) + c * CH * 8,
                        int(colbase[s]) + c * CH * 8 + ntiles * 8,
                    )
                    nc.gpsimd.dma_gather(
                        t[:, 0:ntiles, :],
                        tabs[s],
                        idx_sb[:, sl],
                        ni,
                        ni,
                        OC_PAD,
                        single_packet=(ni <= 1024),
                        queue_num=(gq_counter[0] if CFG.get("QRR") else s) % NQ,
                    )
                    gq_counter[0] += 1
                    msg_tiles[s][c] = t
                    next_call[s] = c + 1

                def emit_oh():
                    k = next_oh[0]
                    npair = min(CHP, pr.n_pairs - k * CHP)
                    t = oh_pool.tile([128, CHP, 128], FP8, tag="oh")
                    nc.sync.dma_start(
                        out=t[:, 0:npair, :],
                        in_=oh_d[:, k * CHP : k * CHP + npair, :],
                    )
                    oh_tiles[k] = t
                    next_oh[0] = k + 1

                for w in range(WIN):
                    seg = pr.segs[w]
                    # make sure resources (plus prefetch) exist
                    for pk in seg:
                        _, s, t = pr.pairs[pk]
                        while next_call[s] <= min(
                            t // CH + CFG.get("PF", 1), calls[s] - 1
                        ):
                            emit_gather(s)
                        while next_oh[0] <= min(pk // CHP + 1, n_oh_chunks - 1):
                            emit_oh()
                    if not do_mm:
                        continue
                    pw = psum_w.tile([128, OUT_C], F32)
                    for j, pk in enumerate(seg):
                        _, s, t = pr.pairs[pk]
                        oh_ap = oh_tiles[pk // CHP][:, pk % CHP, :]
                        msg_ap = msg_tiles[s][t // CH][:, t % CH, 0:OUT_C]
                        nc.tensor.matmul(
                            pw[:],
                            oh_ap,
                            msg_ap,
                            start=(j == 0),
                            stop=(j == len(seg) - 1),
                        )
                    evac(w, pw)
                    if after_window is not None:
                        after_window(w)

            # ---- hop 1:  z1 = (psum + z0) / deg ----
            def evac1(w, pw):
                tmp = tmp_pool.tile([128, OUT_C], F32, tag="tmp")
                nc.vector.tensor_add(tmp[:], pw[:], z0f[:, w, :])
                if CFG["ACT_EVAC"]:
                    nc.scalar.mul(z1f[:, w, :], tmp[:], deginv[:, w : w + 1])
                    nc.scalar.mul(
                        zpad2[:, w, 0:OUT_C], tmp[:], deginv[:, w : w + 1]
                    )
                else:
                    nc.vector.tensor_scalar(
                        z1f[:, w, :], tmp[:], deginv[:, w : w + 1], None, Alu.mult
                    )
                    nc.vector.tensor_copy(zpad2[:, w, 0:OUT_C], z1f[:, w, :])

            agd = CFG.get("AGD", 0)
            ag2_fire = {
                min((c + 1) * CW - 1 + (agd if c < AGC - 1 else 0), WIN - 1): c
                for c in range(AGC)
            }

            def after1(w):
                if STAGE >= 5 and w in ag2_fire:
                    emit_ag(cc2_in, cc2_outs, ag2_fire[w], zpad2)

            if STAGE >= 3:
                run_hop(cc1_outs, evac1, do_mm=STAGE >= 4, after_window=after1)

            # ---- hop 2:  out = dinv * (psum + z1) + b ----
            def evac2(w, pw):
                tmp = tmp_pool.tile([128, OUT_C], F32, tag="tmp")
                tmp2 = tmp_pool.tile([128, OUT_C], F32, tag="tmp2")
                nc.vector.tensor_add(tmp[:], pw[:], z1f[:, w, :])
                if CFG["ACT_EVAC"]:
                    nc.scalar.mul(tmp2[:], tmp[:], dinv[:, w : w + 1])
                else:
                    nc.vector.tensor_scalar(
                        tmp2[:], tmp[:], dinv[:, w : w + 1], None, Alu.mult
                    )
                nc.vector.tensor_add(outst[:, w, :], tmp2[:], b_sb[:])

            def after2(w):
                if (w + 1) % CW == 0:
                    c = w // CW
                    stage_eng.dma_start(
                        out_d[:, c * CW : (c + 1) * CW, :],
                        outst[:, c * CW : (c + 1) * CW, :],
                    )

            if STAGE >= 6:
                run_hop(cc2_outs, evac2, after_window=after2)
            else:
                src_final = {1: z0f, 2: z0f, 3: z0f, 4: z1f, 5: z1f}[STAGE]
                nc.sync.dma_start(out_d[:], src_final[:])

    nc.compile()
    return nc


def _make_in_maps(pr, x, W, b):
    C, NP, WIN = pr.C, pr.NP, pr.WIN
    x = np.asarray(x, dtype=np.float32)
    W = np.asarray(W, dtype=np.float32)
    b = np.asarray(b, dtype=np.float32)
    wt = np.ascontiguousarray(W.T.astype(ml_dtypes.bfloat16))
    b_rep = np.ascontiguousarray(np.broadcast_to(b, (128, len(b))))
    in_maps = []
    for i in range(C):
        xt = np.ascontiguousarray(
            x[i * NP : (i + 1) * NP].T.astype(ml_dtypes.bfloat16)
        )
        in_maps.append(
            dict(
                xt=xt,
                wt=wt,
                bias=b_rep,
                deg=pr.deg_staged[i],
                idx=pr.idx_wrapped[i],
                oh=pr.onehot[i],
            )
        )
    return in_maps


def _unpermute(o, pr):
    # device layout is [p, w, ch]; node order is w*128+p
    return (
        o.reshape(128, pr.WIN, o.shape[-1])
        .transpose(1, 0, 2)
        .reshape(pr.NP, o.shape[-1])
    )


def kernel(x, edge_index, W, b):
    pr = _preprocess(edge_index)
    nc = _build(pr)
    in_maps = _make_in_maps(pr, x, W, b)

    from concourse import bass_utils

    res = bass_utils.run_bass_kernel_spmd(
        nc, in_maps, core_ids=list(range(pr.C))
    )
    shards = [_unpermute(res.results[i]["out"], pr) for i in range(pr.C)]
    return np.ascontiguousarray(np.concatenate(shards, axis=0))
